# revision 41
# baseline (speedup 1.0000x reference)
"""Trainium2 Bass kernel for the AFT-style attention module.

Model (per batch element, S=4096, D=1024, H=16, dh=64):
    q = x@Wq+bq ; k = x@Wk+bk ; v = x@Wv+bv
    aw    = softmax(((q@Wa+ba)*s).T + mask)          # [H,S]
    q_av  = blockdiag(aw @ q)                        # [D]
    p     = k * q_av
    bw    = softmax(((p@Wb+bb)*s).T + mask)          # [H,S]
    p_av  = blockdiag(bw @ p)                        # [D]
    u     = p_av * v
    attn  = (u@Wu+bu + q) @ Wo + bo
    out   = LayerNorm(x + attn) * ln_g + ln_b

Sharding: pure data-parallel - batch B=8 maps 1:1 onto the 8 NeuronCores.

Algebraic restructure (exact, up to fp rounding) - pool x, never q/k:
    ascore = x@(Wq@Wa*s) + (bq@Wa+ba)*s                       [host-folded]
    q_av   = blockdiag((aw@x)@Wq) + bq                        [pool x!]
    bscore = x@(Wk diag(q_av) Wb*s) + (bk diag(q_av) Wb + bb)*s
    p_av   = q_av * (blockdiag((bw@x)@Wk) + bk)
    y      = x@(Mtot + I) + crow,                             [residual folded]
      Mtot = Wv diag(p_av) (Wu@Wo) + Wq@Wo
      crow = (p_av*bv)@(Wu@Wo) + (bq+bu)@Wo + bo
The full q and k tensors are never materialized: no big q/k GEMMs, no
DRAM spills, no DMA-transpose reloads. All heavy matmuls run fp8
DoubleRow with power-of-2 scale management. The dominant x@(Mtot+I)
GEMM uses a 3-term split-precision scheme
    x@M = x8@M8 + xe8@M8/32 + x8@Me8/32   (xe=32(x-x8), Me=32(M-M8))
which measures bit-comparable to bf16 on the final output. LN is
scale-invariant, so the pipeline carries 32*y and never descales.

Two compiled variants:
  - fast: specialized for the observed input pattern (zero mask/biases,
    identity LN affine) - skips mask adds, crow, and the LN affine.
  - general: full algebra for arbitrary inputs.
kernel() checks the actual input values and dispatches.
"""

import os

os.environ.setdefault("MYCRO_LOCAL_CACHE", "1")

import sys

if "/opt/trn_rl_repo" not in sys.path:
    sys.path.insert(0, "/opt/trn_rl_repo")

import numpy as np

S = 4096
D = 1024
H = 16
P = 128
NB = D // P          # 8 d-blocks of 128
SC = 512             # score/pool chunk free dim
NSC = S // SC        # 8
CPB = SC // P        # 4 128-blocks per chunk
NT = S // P          # 32 s-tiles
NPAIR = NT // 2      # 16 DoubleRow s-tile pairs
SCALE = float((D / H) ** -0.5)   # 0.125
EPS = 1e-6
NCORES = 8

# power-of-2 fp8 scale plan
SA = 256.0       # a-score weights
SB = 32768.0     # b-score weights
XN = 32.0        # natural-layout x (also the Mtot working scale)
XE = 32.0        # x fp8-residual
SW = 64.0        # fp8 fold weights (Wq/Wk/WkT/WvT/W1)
SM1 = 65536.0    # m1 = diag(pav) WvT


LAST_EXEC_TIME_NS = None
_COMPILED = {}


def _build_fast():
    """Variant specialized for zero mask/biases + identity LN affine."""
    import concourse.mybir as mybir
    import concourse.tile as tile
    from concourse import bacc
    from concourse.masks import make_identity
    from contextlib import ExitStack

    FP = mybir.dt.float32
    BF = mybir.dt.bfloat16
    F8 = mybir.dt.float8e4
    DR = mybir.MatmulPerfMode.DoubleRow
    AL = mybir.AluOpType
    AF = mybir.ActivationFunctionType

    nc = bacc.Bacc("TRN2", target_bir_lowering=False, debug=False)

    xT8_d = nc.declare_dram_parameter("xT8", [P, NB, S], F8, isOutput=False)
    xeT8_d = nc.declare_dram_parameter("xeT8", [P, NB, S], F8, isOutput=False)
    xn8_d = nc.declare_dram_parameter("xn8", [S, D], F8, isOutput=False)
    W8_d = {
        w: nc.declare_dram_parameter(w, [P, NB, D], F8, isOutput=False)
        for w in ("wq8", "wkT8", "wk8", "wvT8", "w18", "m8h")
    }
    hres_d = nc.declare_dram_parameter("hres16", [P, NB, D], BF,
                                       isOutput=False)
    waq8_d = nc.declare_dram_parameter("waq8", [P, NB, H], F8, isOutput=False)
    wbs_d = nc.declare_dram_parameter("wbs", [P, NB, H], BF, isOutput=False)
    out_d = nc.declare_dram_parameter("out", [S, D], BF, isOutput=True)
    y12_d = nc.dram_tensor("y12", [S, D], BF)

    xn8_r = xn8_d.ap().rearrange("(t p) f -> p t f", p=P)

    with tile.TileContext(nc) as tc, ExitStack() as ctx:
        consts = ctx.enter_context(tc.tile_pool(name="consts", bufs=1))
        wpers = ctx.enter_context(tc.tile_pool(name="wpers", bufs=1))
        sp = ctx.enter_context(tc.tile_pool(name="sp", bufs=2))
        small = ctx.enter_context(tc.tile_pool(name="small", bufs=2))

        id16 = consts.tile([H, H], BF, tag="id16")
        make_identity(nc, id16[:])
        eps_t = consts.tile([P, 1], FP, tag="eps")
        nc.vector.memset(eps_t[:], EPS * XN * XN)
        waq8 = consts.tile([P, NB, H], F8, tag="waq8")
        nc.gpsimd.dma_start(out=waq8[:], in_=waq8_d[:])
        wbs = consts.tile([P, NB, H], BF, tag="wbs")
        nc.gpsimd.dma_start(out=wbs[:], in_=wbs_d[:])

        x8 = wpers.tile([P, NB, S], F8, tag="x8")
        xe8 = wpers.tile([P, NB, S], F8, tag="xe8")
        xn8 = wpers.tile([P, NT, D], F8, tag="xn8")
        me8 = wpers.tile([P, NB, D], F8, tag="me8")
        w8sb = {w: wpers.tile([P, NB, D], F8, tag=w, name=w)
                for w in ("wq8", "wkT8", "wk8", "wvT8", "m8h")}
        m8h = w8sb["m8h"]
        awT8 = consts.tile([P, NT, H], F8, tag="awT8")
        bwT8 = consts.tile([P, NT, H], F8, tag="bwT8")
        asums = consts.tile([H, NSC], FP, tag="asums")
        bsums = consts.tile([H, NSC], FP, tag="bsums")
        qav = consts.tile([P, NB], FP, tag="qav")
        kv = consts.tile([P, NB], FP, tag="kv")
        pav = consts.tile([P, NB], FP, tag="pav")
        pav65 = consts.tile([P, NB], FP, tag="pav65")
        wbq8 = consts.tile([P, NB, H], F8, tag="wbq8")
        awxT8 = consts.tile([P, NB, H], F8, tag="awxT8")
        bwxT8 = consts.tile([P, NB, H], F8, tag="bwxT8")
        wsbT8 = consts.tile([P, NB, H], F8, tag="wsbT8")

        with tc.tile_pool(name="ps_sc", bufs=2, space="PSUM") as ps_sc, \
             tc.tile_pool(name="ps_tp", bufs=2, space="PSUM") as ps_tp, \
             tc.tile_pool(name="ps_pool", bufs=1, space="PSUM") as ps_pool, \
             tc.tile_pool(name="ps_e1", bufs=1, space="PSUM") as ps_e1:

            def e1_tile(t):
                """y12 = 32*(x@M8h) spill: both fp8 x streams, one group"""
                s0 = t * P
                pn = ps_e1.tile([P, 2, SC], FP, tag="e1")
                for half in range(2):
                    hsl = slice(half * SC, (half + 1) * SC)
                    for k2 in range(NB // 2):
                        nc.tensor.matmul(
                            pn[:, half, :],
                            x8[:, 2 * k2:2 * k2 + 2, s0:s0 + P],
                            m8h[:, 2 * k2:2 * k2 + 2, hsl],
                            start=(k2 == 0), stop=False, perf_mode=DR)
                    for k2 in range(NB // 2):
                        nc.tensor.matmul(
                            pn[:, half, :],
                            xe8[:, 2 * k2:2 * k2 + 2, s0:s0 + P],
                            m8h[:, 2 * k2:2 * k2 + 2, hsl],
                            start=False, stop=(k2 == NB // 2 - 1),
                            perf_mode=DR)
                y12t = sp.tile([P, D], BF, tag="y12", bufs=3)
                nc.scalar.activation(y12t[:], pn[:], AF.Copy)
                oeng = (nc.sync, nc.scalar)[t % 2]
                oeng.dma_start(out=y12_d.ap()[s0:s0 + P, :], in_=y12t[:])

            def score_chunk(c, wstat8, sscale, wT_t, sums):
                """exp(x8 @ wstat8 / sscale) -> transposed fp8 weights"""
                lo = c * SC
                ps = ps_sc.tile([H, SC], FP, tag="sc")
                for k2 in range(NB // 2):
                    nc.tensor.matmul(ps[:], wstat8[:, 2 * k2:2 * k2 + 2, :],
                                     x8[:, 2 * k2:2 * k2 + 2, lo:lo + SC],
                                     start=(k2 == 0), stop=(k2 == NB // 2 - 1),
                                     perf_mode=DR)
                awc = sp.tile([H, SC], BF, tag="wc", bufs=2)
                nc.scalar.activation(awc[:], ps[:], AF.Exp,
                                     scale=1.0 / sscale,
                                     accum_out=sums[:, c:c + 1])
                for i in range(CPB):
                    tp = ps_tp.tile([P, H], BF, tag="tp")
                    nc.tensor.matmul(tp[:], awc[:, i * P:(i + 1) * P],
                                     id16[:, :], is_transpose=True)
                    nc.vector.tensor_copy(wT_t[:, c * CPB + i, :], tp[:])

            def pool_chunk(c, wT_t, pool_ps):
                for pr in (2 * c, 2 * c + 1):
                    for half in range(2):
                        nc.tensor.matmul(
                            pool_ps[:, half, :],
                            wT_t[:, 2 * pr:2 * pr + 2, :],
                            xn8[:, 2 * pr:2 * pr + 2,
                                half * SC:(half + 1) * SC],
                            start=(pr == 0), stop=(pr == NPAIR - 1),
                            perf_mode=DR, skip_group_check=True)

            def transposes16(src16, dst, scl):
                """8x [16,128] slices -> dst[:, j, :] fp8, scaled by scl"""
                for j in range(NB):
                    tp = ps_tp.tile([P, H], BF, tag="tp")
                    nc.tensor.matmul(tp[:], src16[:, j * P:(j + 1) * P],
                                     id16[:, :], is_transpose=True)
                    if scl == 1.0:
                        nc.vector.tensor_copy(dst[:, j, :], tp[:])
                    else:
                        nc.vector.tensor_scalar_mul(dst[:, j, :], tp[:], scl)

            def fold16_dr(statT8, w8):
                """[16,D] psum = statT8.T @ w8  (fp8 DR, contract D).
                Shares the ps_pool bank pair - folds and pools alternate."""
                psf = ps_pool.tile([H, 2, SC], FP, tag="plps", name="psf")
                for k2 in range(NB // 2):
                    for half in range(2):
                        nc.tensor.matmul(
                            psf[:, half, :],
                            statT8[:, 2 * k2:2 * k2 + 2, :],
                            w8[:, 2 * k2:2 * k2 + 2,
                               half * SC:(half + 1) * SC],
                            start=(k2 == 0), stop=(k2 == NB // 2 - 1),
                            perf_mode=DR, skip_group_check=True)
                return psf

            def extract_blockdiag(pool_ps, sums, av_t, wtag):
                tot = small.tile([H, 1], FP, tag=f"tot{wtag}")
                nc.vector.reduce_sum(tot[:], sums[:], axis=mybir.AxisListType.X)
                nc.vector.tensor_scalar_mul(tot[:], tot[:], XN)
                rinv = small.tile([H, 1], FP, tag=f"rinv{wtag}")
                nc.vector.reciprocal(rinv[:], tot[:])
                pool_sb = sp.tile([H, D], BF, tag="sb16", bufs=2)
                nc.vector.tensor_scalar_mul(pool_sb[:], pool_ps[:], rinv[:, :1])
                for j in range(NB):
                    tpp = ps_tp.tile([P, H], BF, tag="tp")
                    nc.tensor.matmul(tpp[:], pool_sb[:, j * P:(j + 1) * P],
                                     id16[:, :], is_transpose=True)
                    nc.vector.tensor_copy(
                        av_t[0:64, j:j + 1], tpp[0:64, 2 * j:2 * j + 1])
                    nc.vector.tensor_copy(
                        av_t[64:128, j:j + 1], tpp[64:128, 2 * j + 1:2 * j + 2])

            # ============ Phase A: stream x, a-scores, a-pool ============
            apool_ps = ps_pool.tile([H, 2, SC], FP, tag="plps")
            for c in range(NSC):
                lo = c * SC
                eng = (nc.gpsimd, nc.sync, nc.scalar)[c % 3]
                eng.dma_start(out=x8[:, :, lo:lo + SC],
                              in_=xT8_d[:, :, lo:lo + SC])
                eng2 = (nc.sync, nc.scalar, nc.gpsimd)[c % 3]
                eng2.dma_start(out=xn8[:, 4 * c:4 * c + 4, :],
                               in_=xn8_r[:, 4 * c:4 * c + 4, :])
                # weights wanted from phase B on - issued late in A
                if c >= 6:
                    for cc in (4 * (c - 6), 4 * (c - 6) + 1,
                               4 * (c - 6) + 2, 4 * (c - 6) + 3):
                        nc.scalar.dma_start(out=w8sb["wq8"][:, cc % NB, :],
                                            in_=W8_d["wq8"][:, cc % NB, :])
                        nc.sync.dma_start(out=w8sb["wkT8"][:, cc % NB, :],
                                          in_=W8_d["wkT8"][:, cc % NB, :])
                score_chunk(c, waq8, SA, awT8, asums)
                pool_chunk(c, awT8, apool_ps)

            # xe8/m8h are only consumed by E1 - stream right after phase A
            nc.sync.dma_start(out=m8h[:, 0:4, :], in_=W8_d["m8h"][:, 0:4, :])
            nc.scalar.dma_start(out=m8h[:, 4:8, :], in_=W8_d["m8h"][:, 4:8, :])
            for c in range(NSC):
                lo = c * SC
                eng = (nc.gpsimd, nc.sync, nc.scalar)[c % 3]
                eng.dma_start(out=xe8[:, :, lo:lo + SC],
                              in_=xeT8_d[:, :, lo:lo + SC])
            for c in range(NSC):
                eng = (nc.sync, nc.scalar)[c % 2]
                eng.dma_start(out=w8sb["wk8"][:, c, :],
                              in_=W8_d["wk8"][:, c, :])

            # ============ Phase B: qav fold, wbq, ws_b fold ============
            awx16 = sp.tile([H, D], BF, tag="sb16", bufs=2)
            nc.vector.tensor_copy(awx16[:], apool_ps[:])
            transposes16(awx16, awxT8, 1.0 / SW)
            psq = fold16_dr(awxT8, w8sb["wq8"])
            extract_blockdiag(psq, asums, qav, "a")
            for k in range(NB):
                nc.vector.tensor_scalar_mul(wbq8[:, k, :], wbs[:, k, :],
                                            qav[:, k:k + 1])
            psw = fold16_dr(wbq8, w8sb["wkT8"])
            wsb16 = sp.tile([H, D], BF, tag="sb16", bufs=2)
            nc.vector.tensor_copy(wsb16[:], psw[:])
            transposes16(wsb16, wsbT8, 1.0)
            for t in range(8):
                e1_tile(t)

            # ============ Phase C: b-scores + b-pool + E1 tiles ============
            bpool_ps = ps_pool.tile([H, 2, SC], FP, tag="plps")
            hresc = {}
            for c in range(NSC):
                # prefetch D-phase weights (gpsimd is sacrificial: it may
                # block on the hres tile ring without stalling other work)
                nc.sync.dma_start(out=w8sb["wvT8"][:, c, :],
                                  in_=W8_d["wvT8"][:, c, :])
                w = sp.tile([P, D], BF, tag="hresc", bufs=4, name=f"hres{c}")
                nc.gpsimd.dma_start(out=w[:, 0:SC], in_=hres_d[:, c, 0:SC])
                nc.gpsimd.dma_start(out=w[:, SC:D], in_=hres_d[:, c, SC:D])
                hresc[c] = w
                score_chunk(c, wsbT8, SB * SW, bwT8, bsums)
                pool_chunk(c, bwT8, bpool_ps)
                e1_tile(8 + c)

            # ============ Phase C2: kav fold -> pav ============
            bwx16 = sp.tile([H, D], BF, tag="sb16", bufs=2)
            nc.vector.tensor_copy(bwx16[:], bpool_ps[:])
            transposes16(bwx16, bwxT8, 1.0 / SW)
            psk = fold16_dr(bwxT8, w8sb["wk8"])
            extract_blockdiag(psk, bsums, kv, "b")
            nc.vector.tensor_mul(pav[:], qav[:], kv[:])
            nc.vector.tensor_scalar_mul(pav65[:], pav[:], SM1 / SW)
            for t in range(16, NT):
                e1_tile(t)

        # ============ Phase D: Me8 = fp8(32*(Mtot + I) - up(M8h)) ========
        with tc.tile_pool(name="dtmp", bufs=1) as dtmp, \
             tc.tile_pool(name="ps_mm", bufs=2, space="PSUM") as ps_mm:
            m1T8 = dtmp.tile([P, NB, D], F8, tag="m1T8")
            for k in range(NB):
                nc.scalar.activation(m1T8[:, k, :], w8sb["wvT8"][:, k, :],
                                     AF.Copy, scale=pav65[:, k:k + 1])
            w18 = dtmp.tile([P, NB, D], F8, tag="w18")
            nc.sync.dma_start(out=w18[:, 0:4, :], in_=W8_d["w18"][:, 0:4, :])
            nc.scalar.dma_start(out=w18[:, 4:8, :], in_=W8_d["w18"][:, 4:8, :])
            psc = XN / (SM1 * SW)
            for m in range(NB):
                ps = ps_mm.tile([P, 2, SC], FP, tag="mm")
                for half in range(2):
                    for k2 in range(NB // 2):
                        nc.tensor.matmul(
                            ps[:, half, :],
                            m1T8[:, 2 * k2:2 * k2 + 2, m * P:(m + 1) * P],
                            w18[:, 2 * k2:2 * k2 + 2,
                                half * SC:(half + 1) * SC],
                            start=(k2 == 0), stop=(k2 == NB // 2 - 1),
                            perf_mode=DR)
                m32blk = sp.tile([P, D], BF, tag="m32blk", bufs=2)
                nc.vector.scalar_tensor_tensor(m32blk[:], ps[:], psc,
                                               hresc[m][:], op0=AL.mult,
                                               op1=AL.add)
                nc.scalar.activation(me8[:, m, :], m32blk[:], AF.Copy,
                                     scale=XE)

        # ====== Phase E2: 32*y = y12 + x8@Me8/32 ; LN epilogue ======
        with tc.tile_pool(name="e2sp", bufs=1) as e2sp, \
             tc.tile_pool(name="ps_e2", bufs=2, space="PSUM") as ps_e2:
            for t in range(NT):
                s0 = t * P
                y12r = e2sp.tile([P, D], BF, tag="y12r", bufs=6,
                                 name=f"y12r{t}")
                nc.sync.dma_start(out=y12r[:, 0:SC],
                                  in_=y12_d.ap()[s0:s0 + P, 0:SC])
                nc.scalar.dma_start(out=y12r[:, SC:D],
                                    in_=y12_d.ap()[s0:s0 + P, SC:D])
                pn3 = ps_e2.tile([P, 2, SC], FP, tag="e2")
                for half in range(2):
                    hsl = slice(half * SC, (half + 1) * SC)
                    for k2 in range(NB // 2):
                        nc.tensor.matmul(
                            pn3[:, half, :],
                            x8[:, 2 * k2:2 * k2 + 2, s0:s0 + P],
                            me8[:, 2 * k2:2 * k2 + 2, hsl],
                            start=(k2 == 0), stop=(k2 == NB // 2 - 1),
                            perf_mode=DR)
                ybf = sp.tile([P, D], BF, tag="ybf", bufs=3)
                sy = small.tile([P, 1], FP, tag="sy", bufs=4)
                nc.vector.scalar_tensor_tensor(ybf[:], pn3[:], 1.0 / XN,
                                               y12r[:], op0=AL.mult,
                                               op1=AL.add, accum_out=sy[:])
                sqs = sp.tile([P, D], BF, tag="sqs", bufs=2)
                s2 = small.tile([P, 1], FP, tag="s2", bufs=4)
                nc.scalar.activation(sqs[:], ybf[:], AF.Square,
                                     accum_out=s2[:])
                mu = small.tile([P, 1], FP, tag="mu", bufs=4)
                nc.vector.tensor_scalar_mul(mu[:], sy[:], 1.0 / D)
                mu2 = small.tile([P, 1], FP, tag="mu2", bufs=4)
                nc.vector.tensor_mul(mu2[:], mu[:], mu[:])
                var = small.tile([P, 1], FP, tag="var", bufs=4)
                nc.vector.scalar_tensor_tensor(var[:], s2[:], 1.0 / D,
                                               mu2[:], op0=AL.mult,
                                               op1=AL.subtract)
                sq = small.tile([P, 1], FP, tag="sq", bufs=4)
                nc.scalar.activation(sq[:], var[:], AF.Sqrt,
                                     bias=eps_t[:, :1], scale=1.0)
                rstd = small.tile([P, 1], FP, tag="rstd", bufs=4)
                nc.vector.reciprocal(rstd[:], sq[:])
                nmr = small.tile([P, 1], FP, tag="nmr", bufs=4)
                nc.vector.scalar_tensor_tensor(nmr[:], mu[:], -1.0,
                                               rstd[:], op0=AL.mult,
                                               op1=AL.mult)
                tb = sp.tile([P, D], BF, tag="tb", bufs=3)
                if t % 2 == 0:
                    nc.scalar.activation(tb[:], ybf[:], AF.Identity,
                                         bias=nmr[:, :1], scale=rstd[:, :1])
                else:
                    nc.gpsimd.tensor_scalar(tb[:], ybf[:], rstd[:, :1],
                                            nmr[:, :1], op0=AL.mult,
                                            op1=AL.add)
                oeng = (nc.sync, nc.scalar)[(t + 1) % 2]
                oeng.dma_start(out=out_d[s0:s0 + P, :], in_=tb[:])

    nc.compile()
    return nc


def _host_inputs_fast(x, Wq, Wk, Wv, Wa, Wb, Wu, Wo):
    import ml_dtypes
    BF = ml_dtypes.bfloat16
    F8 = ml_dtypes.float8_e4m3

    def lay(W):
        N = W.shape[1]
        return np.ascontiguousarray(
            W.reshape(NB, P, N).transpose(1, 0, 2)).astype(BF)

    def lay8(W, s):
        return lay(s * W).astype(F8)

    wqoI = lay(XN * (Wq @ Wo + np.eye(D, dtype=np.float32)))
    m8h = wqoI.astype(F8)
    hres16 = (wqoI.astype(np.float32) - m8h.astype(np.float32)).astype(BF)
    weights = {
        "wq8": lay8(Wq, SW),
        "wkT8": lay8(np.ascontiguousarray(Wk.T), SW),
        "wk8": lay8(Wk, SW),
        "wvT8": lay8(np.ascontiguousarray(Wv.T), SW),
        "w18": lay8(Wu @ Wo, SW),
        "m8h": m8h,
        "hres16": hres16,
        "waq8": lay8((Wq @ Wa), SCALE * SA),
        "wbs": lay(Wb * (SCALE * SB)),
    }

    def layx(a):
        return np.ascontiguousarray(a.reshape(S, NB, P).transpose(2, 1, 0))

    def per_core(xi):
        xb = xi.astype(BF)
        xbf = xb.astype(np.float32)
        x8n = xbf.astype(F8)
        # scale 1: subnormal-degraded residual, measured equivalent
        xe8n = (xbf - x8n.astype(np.float32)).astype(F8)
        m = {
            "xT8": layx(x8n),
            "xeT8": layx(xe8n),
            "xn8": (XN * xbf).astype(F8),
        }
        m.update(weights)
        return m

    return per_core


def _build_general():
    """Full-generality variant (arbitrary mask/bias/LN-affine values)."""
    import concourse.mybir as mybir
    import concourse.tile as tile
    from concourse import bacc
    from concourse.masks import make_identity
    from contextlib import ExitStack

    FP = mybir.dt.float32
    BF = mybir.dt.bfloat16
    F8 = mybir.dt.float8e4
    DR = mybir.MatmulPerfMode.DoubleRow
    AL = mybir.AluOpType
    AF = mybir.ActivationFunctionType

    SM1G = 65536.0
    SW1 = 64.0

    nc = bacc.Bacc("TRN2", target_bir_lowering=False, debug=False)

    xT8_d = nc.declare_dram_parameter("xT8", [P, NB, S], F8, isOutput=False)
    xeT8_d = nc.declare_dram_parameter("xeT8", [P, NB, S], F8, isOutput=False)
    xn8_d = nc.declare_dram_parameter("xn8", [S, D], F8, isOutput=False)
    mska_d = nc.declare_dram_parameter("mska", [1, S], BF, isOutput=False)
    mskb_d = nc.declare_dram_parameter("mskb", [1, S], BF, isOutput=False)
    W_d = {
        w: nc.declare_dram_parameter(w, [P, NB, D], BF, isOutput=False)
        for w in ("Wq", "WkT", "Wk", "WvT", "w164", "wqo32")
    }
    waq8_d = nc.declare_dram_parameter("waq8", [P, NB, H], F8, isOutput=False)
    wbs_d = nc.declare_dram_parameter("wbs", [P, NB, H], BF, isOutput=False)
    abias_d = nc.declare_dram_parameter("abias", [H, 1], FP, isOutput=False)
    bbs_d = nc.declare_dram_parameter("bbs", [H, 1], FP, isOutput=False)
    b_d = {
        b: nc.declare_dram_parameter(b, [P, NB], FP, isOutput=False)
        for b in ("bq", "bk", "bv")
    }
    hrow_d = nc.declare_dram_parameter("hrow", [1, D], BF, isOutput=False)
    lng_d = nc.declare_dram_parameter("lng16b", [P, D], BF, isOutput=False)
    lnb_d = nc.declare_dram_parameter("lnb16b", [P, D], BF, isOutput=False)
    out_d = nc.declare_dram_parameter("out", [S, D], BF, isOutput=True)

    xn8_r = xn8_d.ap().rearrange("(t p) f -> p t f", p=P)

    with tile.TileContext(nc) as tc, ExitStack() as ctx:
        consts = ctx.enter_context(tc.tile_pool(name="consts", bufs=1))
        wpers = ctx.enter_context(tc.tile_pool(name="wpers", bufs=1))
        sp = ctx.enter_context(tc.tile_pool(name="sp", bufs=2))
        small = ctx.enter_context(tc.tile_pool(name="small", bufs=2))

        id16 = consts.tile([H, H], BF, tag="id16")
        make_identity(nc, id16[:])
        id32 = consts.tile([P, P], BF, tag="id32")
        make_identity(nc, id32[:])
        nc.vector.tensor_scalar_mul(id32[:], id32[:], XN)
        ones16 = consts.tile([1, H], BF, tag="ones16")
        nc.vector.memset(ones16[:], 1.0)
        eps_t = consts.tile([P, 1], FP, tag="eps")
        nc.vector.memset(eps_t[:], EPS * XN * XN)

        waq8 = consts.tile([P, NB, H], F8, tag="waq8")
        nc.gpsimd.dma_start(out=waq8[:], in_=waq8_d[:])
        wbs = consts.tile([P, NB, H], BF, tag="wbs")
        nc.gpsimd.dma_start(out=wbs[:], in_=wbs_d[:])
        abias = consts.tile([H, 1], FP, tag="abias")
        nc.gpsimd.dma_start(out=abias[:], in_=abias_d[:])
        bbs = consts.tile([H, 1], FP, tag="bbs")
        nc.gpsimd.dma_start(out=bbs[:], in_=bbs_d[:])
        bias_t = {}
        for b in ("bq", "bk", "bv"):
            t = consts.tile([P, NB], FP, tag=f"b_{b}")
            nc.gpsimd.dma_start(out=t[:], in_=b_d[b][:])
            bias_t[b] = t
        bk16 = consts.tile([P, NB], BF, tag="bk16")
        nc.vector.tensor_copy(bk16[:], bias_t["bk"][:])
        lng_b = consts.tile([P, D], BF, tag="lng")
        nc.sync.dma_start(out=lng_b[:], in_=lng_d[:])
        lnb_b = consts.tile([P, D], BF, tag="lnb")
        nc.sync.dma_start(out=lnb_b[:], in_=lnb_d[:])
        hrow = consts.tile([1, D], BF, tag="hrow")
        nc.sync.dma_start(out=hrow[:], in_=hrow_d[:])

        x8 = wpers.tile([P, NB, S], F8, tag="x8")
        xe8 = wpers.tile([P, NB, S], F8, tag="xe8")
        xn8 = wpers.tile([P, NT, D], F8, tag="xn8")
        mtot8 = wpers.tile([P, NB, D], F8, tag="mtot8")
        me8 = wpers.tile([P, NB, D], F8, tag="me8", name="me8")
        awT8 = consts.tile([P, NT, H], F8, tag="awT8")
        bwT8 = consts.tile([P, NT, H], F8, tag="bwT8")
        asums = consts.tile([H, NSC], FP, tag="asums")
        bsums = consts.tile([H, NSC], FP, tag="bsums")
        qav = consts.tile([P, NB], FP, tag="qav")
        kv = consts.tile([P, NB], FP, tag="kv")
        pav = consts.tile([P, NB], FP, tag="pav")
        wbq = consts.tile([P, NB, H], BF, tag="wbq")
        awxT = consts.tile([P, NB, H], BF, tag="awxT")
        bwxT = consts.tile([P, NB, H], BF, tag="bwxT")
        wsbT8 = consts.tile([P, NB, H], F8, tag="wsbT8")
        biasb = consts.tile([H, 1], FP, tag="biasb")
        bvp16 = consts.tile([P, NB], BF, tag="bvp16")
        crow_b = consts.tile([P, D], BF, tag="crowb")

        with tc.tile_pool(name="ps_sc", bufs=2, space="PSUM") as ps_sc, \
             tc.tile_pool(name="ps_tp", bufs=2, space="PSUM") as ps_tp, \
             tc.tile_pool(name="ps_pool", bufs=1, space="PSUM") as ps_pool, \
             tc.tile_pool(name="ps_fold", bufs=1, space="PSUM") as ps_fold:

            def score_chunk(c, wstat8, msk_d, bias_s, sscale, wT_t, sums):
                lo = c * SC
                mc = sp.tile([1, SC], BF, tag="msk", bufs=2)
                nc.sync.dma_start(out=mc[:], in_=msk_d[:, lo:lo + SC])
                ps = ps_sc.tile([H, SC], FP, tag="sc")
                for k2 in range(NB // 2):
                    nc.tensor.matmul(ps[:], wstat8[:, 2 * k2:2 * k2 + 2, :],
                                     x8[:, 2 * k2:2 * k2 + 2, lo:lo + SC],
                                     start=(k2 == 0), stop=False,
                                     perf_mode=DR)
                nc.tensor.matmul(ps[:], ones16[:1, :], mc[:1, :],
                                 start=False, stop=True)
                awc = sp.tile([H, SC], BF, tag="wc", bufs=2)
                nc.scalar.activation(awc[:], ps[:], AF.Exp,
                                     bias=bias_s[:, :1], scale=1.0 / sscale,
                                     accum_out=sums[:, c:c + 1])
                for i in range(CPB):
                    tp = ps_tp.tile([P, H], BF, tag="tp")
                    nc.tensor.matmul(tp[:], awc[:, i * P:(i + 1) * P],
                                     id16[:, :], is_transpose=True)
                    nc.vector.tensor_copy(wT_t[:, c * CPB + i, :], tp[:])

            def pool_chunk(c, wT_t, pool_ps):
                for pr in (2 * c, 2 * c + 1):
                    for half in range(2):
                        nc.tensor.matmul(
                            pool_ps[:, half, :],
                            wT_t[:, 2 * pr:2 * pr + 2, :],
                            xn8[:, 2 * pr:2 * pr + 2,
                                half * SC:(half + 1) * SC],
                            start=(pr == 0), stop=(pr == NPAIR - 1),
                            perf_mode=DR, skip_group_check=True)

            def transpose16(src16, dst, j):
                tp = ps_tp.tile([P, H], BF, tag="tp")
                nc.tensor.matmul(tp[:], src16[:, j * P:(j + 1) * P],
                                 id16[:, :], is_transpose=True)
                nc.vector.tensor_copy(dst[:, j, :], tp[:])

            def fold16(srcT, wname, eng):
                psf = ps_fold.tile([H, 2, SC], FP, tag="fold")
                for k in range(NB):
                    wc = sp.tile([P, D], BF, tag="wfold", bufs=4)
                    eng.dma_start(out=wc[:], in_=W_d[wname][:, k, :])
                    for half in range(2):
                        nc.tensor.matmul(
                            psf[:, half, :], srcT[:, k, :],
                            wc[:, half * SC:(half + 1) * SC],
                            start=(k == 0), stop=(k == NB - 1),
                            skip_group_check=True)
                return psf

            def extract_blockdiag(pool_ps, sums, av_t, bias_av, wtag):
                tot = small.tile([H, 1], FP, tag=f"tot{wtag}")
                nc.vector.reduce_sum(tot[:], sums[:], axis=mybir.AxisListType.X)
                nc.vector.tensor_scalar_mul(tot[:], tot[:], XN)
                rinv = small.tile([H, 1], FP, tag=f"rinv{wtag}")
                nc.vector.reciprocal(rinv[:], tot[:])
                pool_sb = sp.tile([H, D], BF, tag="sb16", bufs=2)
                nc.vector.tensor_scalar_mul(pool_sb[:], pool_ps[:], rinv[:, :1])
                for j in range(NB):
                    tpp = ps_tp.tile([P, H], BF, tag="tp")
                    nc.tensor.matmul(tpp[:], pool_sb[:, j * P:(j + 1) * P],
                                     id16[:, :], is_transpose=True)
                    nc.vector.tensor_copy(
                        av_t[0:64, j:j + 1], tpp[0:64, 2 * j:2 * j + 1])
                    nc.vector.tensor_copy(
                        av_t[64:128, j:j + 1], tpp[64:128, 2 * j + 1:2 * j + 2])
                nc.vector.tensor_add(av_t[:], av_t[:], bias_av[:])

            apool_ps = ps_pool.tile([H, 2, SC], FP, tag="plps")
            for c in range(NSC):
                lo = c * SC
                nc.gpsimd.dma_start(out=x8[:, :, lo:lo + SC],
                                    in_=xT8_d[:, :, lo:lo + SC])
                nc.scalar.dma_start(out=xn8[:, 4 * c:4 * c + 4, :],
                                    in_=xn8_r[:, 4 * c:4 * c + 4, :])
                score_chunk(c, waq8, mska_d, abias, SA, awT8, asums)
                pool_chunk(c, awT8, apool_ps)
            for c in range(NSC):
                lo = c * SC
                eng = (nc.gpsimd, nc.scalar)[c % 2]
                eng.dma_start(out=xe8[:, :, lo:lo + SC],
                              in_=xeT8_d[:, :, lo:lo + SC])

            awx16 = sp.tile([H, D], BF, tag="sb16", bufs=2)
            nc.vector.tensor_copy(awx16[:], apool_ps[:])
            for j in range(NB):
                transpose16(awx16, awxT, j)
            psq = fold16(awxT, "Wq", nc.sync)
            extract_blockdiag(psq, asums, qav, bias_t["bq"], "a")
            for k in range(NB):
                nc.vector.tensor_scalar_mul(wbq[:, k, :], wbs[:, k, :],
                                            qav[:, k:k + 1])
            psw = fold16(wbq, "WkT", nc.sync)
            wsb16 = sp.tile([H, D], BF, tag="sb16", bufs=2)
            nc.vector.tensor_copy(wsb16[:], psw[:])
            for j in range(NB):
                transpose16(wsb16, wsbT8, j)
            psbb = ps_fold.tile([H, 1], FP, tag="fold", name="psbb")
            for k in range(NB):
                nc.tensor.matmul(psbb[:], wbq[:, k, :], bk16[:, k:k + 1],
                                 start=(k == 0), stop=(k == NB - 1))
            nc.vector.scalar_tensor_tensor(biasb[:], psbb[:], 1.0 / SB,
                                           bbs[:], op0=AL.mult, op1=AL.add)

            bpool_ps = ps_pool.tile([H, 2, SC], FP, tag="plps")
            for c in range(NSC):
                score_chunk(c, wsbT8, mskb_d, biasb, SB, bwT8, bsums)
                pool_chunk(c, bwT8, bpool_ps)

            bwx16 = sp.tile([H, D], BF, tag="sb16", bufs=2)
            nc.vector.tensor_copy(bwx16[:], bpool_ps[:])
            for j in range(NB):
                transpose16(bwx16, bwxT, j)
            psk = fold16(bwxT, "Wk", nc.sync)
            extract_blockdiag(psk, bsums, kv, bias_t["bk"], "b")
            nc.vector.tensor_mul(pav[:], qav[:], kv[:])
            bvp = small.tile([P, NB], FP, tag="bvp")
            nc.vector.tensor_mul(bvp[:], bias_t["bv"][:], pav[:])
            nc.vector.tensor_copy(bvp16[:], bvp[:])

        with tc.tile_pool(name="dtmp", bufs=1) as dtmp, \
             tc.tile_pool(name="ps_mm", bufs=2, space="PSUM") as ps_mm, \
             tc.tile_pool(name="ps_cr", bufs=1, space="PSUM") as ps_cr:
            pav65 = small.tile([P, NB], FP, tag="pav65")
            nc.vector.tensor_scalar_mul(pav65[:], pav[:], SM1G)
            m1T8 = dtmp.tile([P, NB, D], F8, tag="m1T8")
            for k in range(NB):
                wvc = sp.tile([P, D], BF, tag="wfold", bufs=4)
                nc.sync.dma_start(out=wvc[:], in_=W_d["WvT"][:, k, :])
                nc.scalar.activation(m1T8[:, k, :], wvc[:], AF.Copy,
                                     scale=pav65[:, k:k + 1])
            w18 = dtmp.tile([P, NB, D], F8, tag="w18")
            cr_ps = ps_cr.tile([H, 2, SC], FP, tag="cr")
            for k in range(NB):
                w1c = sp.tile([P, D], BF, tag="wfold", bufs=4)
                nc.sync.dma_start(out=w1c[:], in_=W_d["w164"][:, k, :])
                nc.scalar.activation(w18[:, k, :], w1c[:], AF.Copy)
                for half in range(2):
                    nc.tensor.matmul(
                        cr_ps[0:1, half, :], bvp16[:, k:k + 1],
                        w1c[:, half * SC:(half + 1) * SC],
                        start=(k == 0), stop=(k == NB - 1),
                        skip_group_check=True)
            crow16 = sp.tile([1, D], BF, tag="sb16", bufs=2)
            nc.vector.scalar_tensor_tensor(
                crow16[:1, :], cr_ps[0:1, :], XN / SW1, hrow[:1, :],
                op0=AL.mult, op1=AL.add)
            nc.gpsimd.partition_broadcast(crow_b[:], crow16[:1, :])

            psc = XN / (SM1G * SW1)
            for m in range(NB):
                ps = ps_mm.tile([P, 2, SC], FP, tag="mm")
                for half in range(2):
                    for k2 in range(NB // 2):
                        nc.tensor.matmul(
                            ps[:, half, :],
                            m1T8[:, 2 * k2:2 * k2 + 2, m * P:(m + 1) * P],
                            w18[:, 2 * k2:2 * k2 + 2,
                                half * SC:(half + 1) * SC],
                            start=(k2 == 0), stop=(k2 == NB // 2 - 1),
                            perf_mode=DR)
                wqoc = sp.tile([P, D], BF, tag="wqoc", bufs=2)
                nc.sync.dma_start(out=wqoc[:], in_=W_d["wqo32"][:, m, :])
                m32blk = sp.tile([P, D], BF, tag="m32blk", bufs=2)
                nc.vector.scalar_tensor_tensor(m32blk[:], ps[:], psc,
                                               wqoc[:], op0=AL.mult,
                                               op1=AL.add)
                nc.vector.tensor_add(m32blk[:, m * P:(m + 1) * P],
                                     m32blk[:, m * P:(m + 1) * P], id32[:])
                nc.scalar.activation(mtot8[:, m, :], m32blk[:], AF.Copy)
                up16 = sp.tile([P, D], BF, tag="up16", bufs=2)
                nc.vector.tensor_copy(up16[:], mtot8[:, m, :])
                me32 = sp.tile([P, D], BF, tag="me32", bufs=2)
                nc.vector.tensor_sub(me32[:], m32blk[:], up16[:])
                nc.scalar.activation(me8[:, m, :], me32[:], AF.Copy,
                                     scale=XE)

        with tc.tile_pool(name="ps_e1", bufs=2, space="PSUM") as ps_e1, \
             tc.tile_pool(name="ps_e2", bufs=2, space="PSUM") as ps_e2:
            for t in range(NT):
                s0 = t * P
                pn1 = ps_e1.tile([P, 2, SC], FP, tag="e1")
                pn2 = ps_e2.tile([P, 2, SC], FP, tag="e2")
                for half in range(2):
                    hsl = slice(half * SC, (half + 1) * SC)
                    for k2 in range(NB // 2):
                        nc.tensor.matmul(
                            pn1[:, half, :],
                            x8[:, 2 * k2:2 * k2 + 2, s0:s0 + P],
                            mtot8[:, 2 * k2:2 * k2 + 2, hsl],
                            start=(k2 == 0), stop=(k2 == NB // 2 - 1),
                            perf_mode=DR)
                    for k2 in range(NB // 2):
                        nc.tensor.matmul(
                            pn2[:, half, :],
                            xe8[:, 2 * k2:2 * k2 + 2, s0:s0 + P],
                            mtot8[:, 2 * k2:2 * k2 + 2, hsl],
                            start=(k2 == 0), stop=False, perf_mode=DR)
                    for k2 in range(NB // 2):
                        nc.tensor.matmul(
                            pn2[:, half, :],
                            x8[:, 2 * k2:2 * k2 + 2, s0:s0 + P],
                            me8[:, 2 * k2:2 * k2 + 2, hsl],
                            start=False, stop=(k2 == NB // 2 - 1),
                            perf_mode=DR)
                t28 = sp.tile([P, D], BF, tag="t28", bufs=2)
                nc.scalar.activation(t28[:], pn2[:], AF.Copy,
                                     scale=1.0 / XN)
                nc.vector.tensor_add(t28[:], t28[:], crow_b[:])
                ybf = sp.tile([P, D], BF, tag="ybf", bufs=2)
                nc.vector.scalar_tensor_tensor(ybf[:], pn1[:], 1.0,
                                               t28[:], op0=AL.mult,
                                               op1=AL.add)
                stats = small.tile([P, 2, 6], FP, tag="stats")
                nc.vector.bn_stats(stats[:, 0, :], ybf[:, 0:SC])
                nc.vector.bn_stats(stats[:, 1, :], ybf[:, SC:D])
                mv = small.tile([P, 2], FP, tag="mv")
                nc.vector.bn_aggr(mv[:], stats[:])
                sq = small.tile([P, 1], FP, tag="sq")
                nc.scalar.activation(sq[:], mv[:, 1:2], AF.Sqrt,
                                     bias=eps_t[:, :1], scale=1.0)
                rstd = small.tile([P, 1], FP, tag="rstd")
                nc.vector.reciprocal(rstd[:], sq[:])
                nmr = small.tile([P, 1], FP, tag="nmr")
                nc.vector.scalar_tensor_tensor(nmr[:], mv[:, 0:1], -1.0,
                                               rstd[:], op0=AL.mult,
                                               op1=AL.mult)
                tb = sp.tile([P, D], BF, tag="tb", bufs=3)
                nc.scalar.activation(tb[:], ybf[:], AF.Identity,
                                     bias=nmr[:, :1], scale=rstd[:, :1])
                nc.vector.tensor_mul(tb[:], tb[:], lng_b[:])
                nc.gpsimd.tensor_add(tb[:], tb[:], lnb_b[:])
                oeng = (nc.sync, nc.scalar)[(t + 1) % 2]
                oeng.dma_start(out=out_d[s0:s0 + P, :], in_=tb[:])

    nc.compile()
    return nc


def _install_ntff_hook_shim():
    """The agent image's antenv lacks axon_hooks, so trace=True degrades.
    Recreate the hook from the boot helper so neuron-profile works."""
    import types
    try:
        import antenv.axon_hooks  # noqa: F401
        return
    except ImportError:
        pass
    try:
        import antenv
        from trn_agent_boot.trn_boot import _ntff_profile_via_ctypes
        hook = _ntff_profile_via_ctypes("/opt/axon/libaxon_pjrt.so")
        mod = types.ModuleType("antenv.axon_hooks")
        mod._hook = hook
        mod.get_axon_ntff_profile_hook = lambda: mod._hook
        mod.set_axon_ntff_profile_hook = lambda h: setattr(mod, "_hook", h)
        sys.modules["antenv.axon_hooks"] = mod
        antenv.axon_hooks = mod
    except Exception as e:  # tracing is best-effort
        print(f"ntff hook shim failed: {e}", file=sys.stderr)


def kernel(x, mask, Wq, bq, Wk, bk, Wv, bv, Wa, ba, Wb, bb, Wu, bu, Wo, bo,
           ln_g, ln_b):
    global LAST_EXEC_TIME_NS
    import ml_dtypes
    from concourse.bass_utils import run_bass_kernel_spmd

    BF = ml_dtypes.bfloat16
    F8 = ml_dtypes.float8_e4m3
    f32 = lambda a: np.ascontiguousarray(np.asarray(a, dtype=np.float32))

    x = f32(x)
    B = x.shape[0]
    assert B == NCORES and x.shape == (B, S, D)
    mask = f32(mask).reshape(B, S)
    Wq, Wk, Wv, Wu, Wo = f32(Wq), f32(Wk), f32(Wv), f32(Wu), f32(Wo)
    Wa, Wb = f32(Wa), f32(Wb)
    bq, bk, bv, ba, bb, bu, bo = map(f32, (bq, bk, bv, ba, bb, bu, bo))
    ln_g, ln_b = f32(ln_g), f32(ln_b)

    plain = (not mask.any()) and all(
        not a.any() for a in (bq, bk, bv, ba, bb, bu, bo, ln_b)) and bool(
        np.all(ln_g == 1.0))

    trace = bool(int(os.environ.get("KERNEL_TRACE", "0")))
    if trace:
        _install_ntff_hook_shim()

    if plain:
        if "fast" not in _COMPILED:
            _COMPILED["fast"] = _build_fast()
        nc = _COMPILED["fast"]
        per_core = _host_inputs_fast(x, Wq, Wk, Wv, Wa, Wb, Wu, Wo)
        in_maps = [per_core(x[i]) for i in range(B)]
    else:
        if "gen" not in _COMPILED:
            _COMPILED["gen"] = _build_general()
        nc = _COMPILED["gen"]

        def lay(W):
            N = W.shape[1]
            return np.ascontiguousarray(
                W.reshape(NB, P, N).transpose(1, 0, 2)).astype(BF)

        W1f = Wu @ Wo
        weights = {
            "Wq": lay(Wq), "WkT": lay(np.ascontiguousarray(Wk.T)),
            "Wk": lay(Wk), "WvT": lay(np.ascontiguousarray(Wv.T)),
            "w164": lay(64.0 * W1f), "wqo32": lay(XN * (Wq @ Wo)),
            "waq8": lay((Wq @ Wa) * (SCALE * SA)).astype(F8),
            "wbs": lay(Wb * (SCALE * SB)),
        }
        smalls = {
            "abias": (((bq @ Wa) + ba) * SCALE).reshape(H, 1),
            "bbs": (bb * SCALE).reshape(H, 1),
            "bq": np.ascontiguousarray(bq.reshape(NB, P).T),
            "bk": np.ascontiguousarray(bk.reshape(NB, P).T),
            "bv": np.ascontiguousarray(bv.reshape(NB, P).T),
            "hrow": (XN * ((bq + bu) @ Wo + bo)).reshape(1, D).astype(BF),
            "lng16b": np.ascontiguousarray(
                np.broadcast_to(ln_g.reshape(1, D), (P, D))).astype(BF),
            "lnb16b": np.ascontiguousarray(
                np.broadcast_to(ln_b.reshape(1, D), (P, D))).astype(BF),
        }

        def layx(a):
            return np.ascontiguousarray(
                a.reshape(S, NB, P).transpose(2, 1, 0))

        in_maps = []
        for i in range(B):
            xb = x[i].astype(BF)
            xbf = xb.astype(np.float32)
            x8n = xbf.astype(F8)
            xe8n = ((xbf - x8n.astype(np.float32)) * XE).astype(F8)
            m = {
                "xT8": layx(x8n),
                "xeT8": layx(xe8n),
                "xn8": (XN * xbf).astype(F8),
                "mska": (SA * mask[i:i + 1]).astype(BF),
                "mskb": (SB * mask[i:i + 1]).astype(BF),
            }
            m.update(weights)
            m.update(smalls)
            in_maps.append(m)

    res = run_bass_kernel_spmd(nc, in_maps, core_ids=list(range(NCORES)),
                               trace=trace)
    LAST_EXEC_TIME_NS = res.exec_time_ns
    out = np.stack([np.asarray(res.results[i]["out"]).astype(np.float32)
                    for i in range(B)], axis=0)
    return out


if __name__ == "__main__":
    np.random.seed(0)
    ins = {
        "x": np.random.randn(NCORES, S, D).astype(np.float32),
        "mask": np.zeros((NCORES, 1, S), np.float32),
    }
    std = 0.02
    for n, shp in (("Wq", (D, D)), ("Wk", (D, D)), ("Wv", (D, D)),
                   ("Wa", (D, H)), ("Wb", (D, H)), ("Wu", (D, D)),
                   ("Wo", (D, D))):
        ins[n] = (std * np.random.randn(*shp)).astype(np.float32)
    for n, shp in (("bq", (D,)), ("bk", (D,)), ("bv", (D,)), ("ba", (H,)),
                   ("bb", (H,)), ("bu", (D,)), ("bo", (D,)), ("ln_b", (D,))):
        ins[n] = np.zeros(shp, np.float32)
    ins["ln_g"] = np.ones((D,), np.float32)
    out = kernel(**ins)
    print("out", out.shape, out.dtype, float(np.abs(out).mean()))


# revision 42
# speedup vs baseline: 1.0675x; 1.0675x over previous
"""Trainium2 Bass kernel for the AFT-style attention module.

Model (per batch element, S=4096, D=1024, H=16, dh=64):
    q = x@Wq+bq ; k = x@Wk+bk ; v = x@Wv+bv
    aw    = softmax(((q@Wa+ba)*s).T + mask)          # [H,S]
    q_av  = blockdiag(aw @ q)                        # [D]
    p     = k * q_av
    bw    = softmax(((p@Wb+bb)*s).T + mask)          # [H,S]
    p_av  = blockdiag(bw @ p)                        # [D]
    u     = p_av * v
    attn  = (u@Wu+bu + q) @ Wo + bo
    out   = LayerNorm(x + attn) * ln_g + ln_b

Sharding: pure data-parallel - batch B=8 maps 1:1 onto the 8 NeuronCores.

Algebraic restructure (exact, up to fp rounding) - pool x, never q/k:
    ascore = x@(Wq@Wa*s) + (bq@Wa+ba)*s                       [host-folded]
    q_av   = blockdiag((aw@x)@Wq) + bq                        [pool x!]
    bscore = x@(Wk diag(q_av) Wb*s) + (bk diag(q_av) Wb + bb)*s
    p_av   = q_av * (blockdiag((bw@x)@Wk) + bk)
    y      = x@(Mtot + I) + crow,                             [residual folded]
      Mtot = Wv diag(p_av) (Wu@Wo) + Wq@Wo
      crow = (p_av*bv)@(Wu@Wo) + (bq+bu)@Wo + bo
The full q and k tensors are never materialized: no big q/k GEMMs, no
DRAM spills, no DMA-transpose reloads. All heavy matmuls run fp8
DoubleRow with power-of-2 scale management. The dominant x@(Mtot+I)
GEMM uses a 3-term split-precision scheme
    x@M = x8@M8 + xe8@M8/32 + x8@Me8/32   (xe=32(x-x8), Me=32(M-M8))
which measures bit-comparable to bf16 on the final output. LN is
scale-invariant, so the pipeline carries 32*y and never descales.

Two compiled variants:
  - fast: specialized for the observed input pattern (zero mask/biases,
    identity LN affine) - skips mask adds, crow, and the LN affine.
  - general: full algebra for arbitrary inputs.
kernel() checks the actual input values and dispatches.
"""

import os

os.environ.setdefault("MYCRO_LOCAL_CACHE", "1")

import sys

if "/opt/trn_rl_repo" not in sys.path:
    sys.path.insert(0, "/opt/trn_rl_repo")

import numpy as np

S = 4096
D = 1024
H = 16
P = 128
NB = D // P          # 8 d-blocks of 128
SC = 512             # score/pool chunk free dim
NSC = S // SC        # 8
CPB = SC // P        # 4 128-blocks per chunk
NT = S // P          # 32 s-tiles
NPAIR = NT // 2      # 16 DoubleRow s-tile pairs
SCALE = float((D / H) ** -0.5)   # 0.125
EPS = 1e-6
NCORES = 8

# power-of-2 fp8 scale plan
SA = 256.0       # a-score weights
SB = 32768.0     # b-score weights
XN = 32.0        # natural-layout x (also the Mtot working scale)
XE = 32.0        # x fp8-residual
SW = 64.0        # fp8 fold weights (Wq/Wk/WkT/WvT/W1)
SM1 = 65536.0    # m1 = diag(pav) WvT


LAST_EXEC_TIME_NS = None
_COMPILED = {}


def _build_fast():
    """Variant specialized for zero mask/biases + identity LN affine."""
    import concourse.mybir as mybir
    import concourse.tile as tile
    from concourse import bacc
    from concourse.masks import make_identity
    from contextlib import ExitStack

    FP = mybir.dt.float32
    BF = mybir.dt.bfloat16
    F8 = mybir.dt.float8e4
    DR = mybir.MatmulPerfMode.DoubleRow
    AL = mybir.AluOpType
    AF = mybir.ActivationFunctionType

    nc = bacc.Bacc("TRN2", target_bir_lowering=False, debug=False)

    xT8_d = nc.declare_dram_parameter("xT8", [P, NB, S], F8, isOutput=False)
    xeT8_d = nc.declare_dram_parameter("xeT8", [P, NB, S], F8, isOutput=False)
    xn8_d = nc.declare_dram_parameter("xn8", [S, D], F8, isOutput=False)
    W8_d = {
        w: nc.declare_dram_parameter(w, [P, NB, D], F8, isOutput=False)
        for w in ("wq8", "wkT8", "wk8", "wvT8", "w18", "m8h")
    }
    hres_d = nc.declare_dram_parameter("hres16", [P, NB, D], BF,
                                       isOutput=False)
    waq8_d = nc.declare_dram_parameter("waq8", [P, NB, H], F8, isOutput=False)
    wbs_d = nc.declare_dram_parameter("wbs", [P, NB, H], BF, isOutput=False)
    out_d = nc.declare_dram_parameter("out", [S, D], BF, isOutput=True)
    y12_d = nc.dram_tensor("y12", [S, D], BF)

    xn8_r = xn8_d.ap().rearrange("(t p) f -> p t f", p=P)

    with tile.TileContext(nc) as tc, ExitStack() as ctx:
        consts = ctx.enter_context(tc.tile_pool(name="consts", bufs=1))
        wpers = ctx.enter_context(tc.tile_pool(name="wpers", bufs=1))
        sp = ctx.enter_context(tc.tile_pool(name="sp", bufs=2))
        small = ctx.enter_context(tc.tile_pool(name="small", bufs=2))

        id16 = consts.tile([H, H], BF, tag="id16")
        make_identity(nc, id16[:])
        eps_t = consts.tile([P, 1], FP, tag="eps")
        nc.vector.memset(eps_t[:], EPS * XN * XN)
        waq8 = consts.tile([P, NB, H], F8, tag="waq8")
        nc.gpsimd.dma_start(out=waq8[:], in_=waq8_d[:])
        wbs = consts.tile([P, NB, H], BF, tag="wbs")
        nc.gpsimd.dma_start(out=wbs[:], in_=wbs_d[:])

        x8 = wpers.tile([P, NB, S], F8, tag="x8")
        xe8 = wpers.tile([P, NB, S], F8, tag="xe8")
        me8 = wpers.tile([P, NB, D], F8, tag="me8")
        ykeep = ctx.enter_context(tc.tile_pool(name="ykeep", bufs=1))
        ykt = {}
        w8sb = {w: wpers.tile([P, NB, D], F8, tag=w, name=w)
                for w in ("wq8", "wkT8", "wk8", "wvT8", "m8h")}
        m8h = w8sb["m8h"]
        awT8 = consts.tile([P, NT, H], F8, tag="awT8")
        bwT8 = consts.tile([P, NT, H], F8, tag="bwT8")
        asums = consts.tile([H, NSC], FP, tag="asums")
        bsums = consts.tile([H, NSC], FP, tag="bsums")
        qav = consts.tile([P, NB], FP, tag="qav")
        kv = consts.tile([P, NB], FP, tag="kv")
        pav = consts.tile([P, NB], FP, tag="pav")
        pav65 = consts.tile([P, NB], FP, tag="pav65")
        wbq8 = consts.tile([P, NB, H], F8, tag="wbq8")
        awxT8 = consts.tile([P, NB, H], F8, tag="awxT8")
        bwxT8 = consts.tile([P, NB, H], F8, tag="bwxT8")
        wsbT8 = consts.tile([P, NB, H], F8, tag="wsbT8")

        with tc.tile_pool(name="xnp", bufs=1) as xnp, \
             tc.tile_pool(name="ps_sc", bufs=2, space="PSUM") as ps_sc, \
             tc.tile_pool(name="ps_tp", bufs=2, space="PSUM") as ps_tp, \
             tc.tile_pool(name="ps_pool", bufs=1, space="PSUM") as ps_pool, \
             tc.tile_pool(name="ps_e1", bufs=1, space="PSUM") as ps_e1:

            xn8 = xnp.tile([P, NT, D], F8, tag="xn8")

            def e1_tile(t):
                """y12 = 32*(x@M8h) spill: both fp8 x streams, one group"""
                s0 = t * P
                pn = ps_e1.tile([P, 2, SC], FP, tag="e1")
                for half in range(2):
                    hsl = slice(half * SC, (half + 1) * SC)
                    for k2 in range(NB // 2):
                        nc.tensor.matmul(
                            pn[:, half, :],
                            x8[:, 2 * k2:2 * k2 + 2, s0:s0 + P],
                            m8h[:, 2 * k2:2 * k2 + 2, hsl],
                            start=(k2 == 0), stop=False, perf_mode=DR)
                    for k2 in range(NB // 2):
                        nc.tensor.matmul(
                            pn[:, half, :],
                            xe8[:, 2 * k2:2 * k2 + 2, s0:s0 + P],
                            m8h[:, 2 * k2:2 * k2 + 2, hsl],
                            start=False, stop=(k2 == NB // 2 - 1),
                            perf_mode=DR)
                if t >= NT - 8:
                    yk = ykeep.tile([P, D], BF, tag=f"yk{t}",
                                    name=f"yk{t}")
                    nc.scalar.activation(yk[:], pn[:], AF.Copy)
                    ykt[t] = yk
                else:
                    y12t = sp.tile([P, D], BF, tag="y12", bufs=3)
                    nc.scalar.activation(y12t[:], pn[:], AF.Copy)
                    oeng = (nc.sync, nc.scalar)[t % 2]
                    oeng.dma_start(out=y12_d.ap()[s0:s0 + P, :],
                                   in_=y12t[:])

            def score_chunk(c, wstat8, sscale, wT_t, sums):
                """exp(x8 @ wstat8 / sscale) -> transposed fp8 weights"""
                lo = c * SC
                ps = ps_sc.tile([H, SC], FP, tag="sc")
                for k2 in range(NB // 2):
                    nc.tensor.matmul(ps[:], wstat8[:, 2 * k2:2 * k2 + 2, :],
                                     x8[:, 2 * k2:2 * k2 + 2, lo:lo + SC],
                                     start=(k2 == 0), stop=(k2 == NB // 2 - 1),
                                     perf_mode=DR)
                awc = sp.tile([H, SC], BF, tag="wc", bufs=2)
                nc.scalar.activation(awc[:], ps[:], AF.Exp,
                                     scale=1.0 / sscale,
                                     accum_out=sums[:, c:c + 1])
                for i in range(CPB):
                    tp = ps_tp.tile([P, H], BF, tag="tp")
                    nc.tensor.matmul(tp[:], awc[:, i * P:(i + 1) * P],
                                     id16[:, :], is_transpose=True)
                    nc.vector.tensor_copy(wT_t[:, c * CPB + i, :], tp[:])

            def pool_chunk(c, wT_t, pool_ps):
                for pr in (2 * c, 2 * c + 1):
                    for half in range(2):
                        nc.tensor.matmul(
                            pool_ps[:, half, :],
                            wT_t[:, 2 * pr:2 * pr + 2, :],
                            xn8[:, 2 * pr:2 * pr + 2,
                                half * SC:(half + 1) * SC],
                            start=(pr == 0), stop=(pr == NPAIR - 1),
                            perf_mode=DR, skip_group_check=True)

            def transposes16(src16, dst, scl):
                """8x [16,128] slices -> dst[:, j, :] fp8, scaled by scl"""
                for j in range(NB):
                    tp = ps_tp.tile([P, H], BF, tag="tp")
                    nc.tensor.matmul(tp[:], src16[:, j * P:(j + 1) * P],
                                     id16[:, :], is_transpose=True)
                    if scl == 1.0:
                        nc.vector.tensor_copy(dst[:, j, :], tp[:])
                    else:
                        nc.vector.tensor_scalar_mul(dst[:, j, :], tp[:], scl)

            def fold16_dr(statT8, w8):
                """[16,D] psum = statT8.T @ w8  (fp8 DR, contract D).
                Shares the ps_pool bank pair - folds and pools alternate."""
                psf = ps_pool.tile([H, 2, SC], FP, tag="plps", name="psf")
                for k2 in range(NB // 2):
                    for half in range(2):
                        nc.tensor.matmul(
                            psf[:, half, :],
                            statT8[:, 2 * k2:2 * k2 + 2, :],
                            w8[:, 2 * k2:2 * k2 + 2,
                               half * SC:(half + 1) * SC],
                            start=(k2 == 0), stop=(k2 == NB // 2 - 1),
                            perf_mode=DR, skip_group_check=True)
                return psf

            def extract_blockdiag(pool_ps, sums, av_t, wtag):
                tot = small.tile([H, 1], FP, tag=f"tot{wtag}")
                nc.vector.reduce_sum(tot[:], sums[:], axis=mybir.AxisListType.X)
                nc.vector.tensor_scalar_mul(tot[:], tot[:], XN)
                rinv = small.tile([H, 1], FP, tag=f"rinv{wtag}")
                nc.vector.reciprocal(rinv[:], tot[:])
                pool_sb = sp.tile([H, D], BF, tag="sb16", bufs=2)
                nc.vector.tensor_scalar_mul(pool_sb[:], pool_ps[:], rinv[:, :1])
                for j in range(NB):
                    tpp = ps_tp.tile([P, H], BF, tag="tp")
                    nc.tensor.matmul(tpp[:], pool_sb[:, j * P:(j + 1) * P],
                                     id16[:, :], is_transpose=True)
                    nc.vector.tensor_copy(
                        av_t[0:64, j:j + 1], tpp[0:64, 2 * j:2 * j + 1])
                    nc.vector.tensor_copy(
                        av_t[64:128, j:j + 1], tpp[64:128, 2 * j + 1:2 * j + 2])

            # ============ Phase A: stream x, a-scores, a-pool ============
            apool_ps = ps_pool.tile([H, 2, SC], FP, tag="plps")
            for c in range(NSC):
                lo = c * SC
                eng = (nc.gpsimd, nc.sync, nc.scalar)[c % 3]
                eng.dma_start(out=x8[:, :, lo:lo + SC],
                              in_=xT8_d[:, :, lo:lo + SC])
                eng2 = (nc.sync, nc.scalar, nc.gpsimd)[c % 3]
                eng2.dma_start(out=xn8[:, 4 * c:4 * c + 4, :],
                               in_=xn8_r[:, 4 * c:4 * c + 4, :])
                # weights wanted from phase B on - issued late in A
                if c >= 6:
                    for cc in (4 * (c - 6), 4 * (c - 6) + 1,
                               4 * (c - 6) + 2, 4 * (c - 6) + 3):
                        nc.scalar.dma_start(out=w8sb["wq8"][:, cc % NB, :],
                                            in_=W8_d["wq8"][:, cc % NB, :])
                        nc.sync.dma_start(out=w8sb["wkT8"][:, cc % NB, :],
                                          in_=W8_d["wkT8"][:, cc % NB, :])
                score_chunk(c, waq8, SA, awT8, asums)
                pool_chunk(c, awT8, apool_ps)

            # xe8/m8h are only consumed by E1 - stream right after phase A
            nc.sync.dma_start(out=m8h[:, 0:4, :], in_=W8_d["m8h"][:, 0:4, :])
            nc.scalar.dma_start(out=m8h[:, 4:8, :], in_=W8_d["m8h"][:, 4:8, :])
            for c in range(NSC):
                lo = c * SC
                eng = (nc.gpsimd, nc.sync, nc.scalar)[c % 3]
                eng.dma_start(out=xe8[:, :, lo:lo + SC],
                              in_=xeT8_d[:, :, lo:lo + SC])
            for c in range(NSC):
                eng = (nc.sync, nc.scalar)[c % 2]
                eng.dma_start(out=w8sb["wk8"][:, c, :],
                              in_=W8_d["wk8"][:, c, :])

            # ============ Phase B: qav fold, wbq, ws_b fold ============
            awx16 = sp.tile([H, D], BF, tag="sb16", bufs=2)
            nc.vector.tensor_copy(awx16[:], apool_ps[:])
            transposes16(awx16, awxT8, 1.0 / SW)
            psq = fold16_dr(awxT8, w8sb["wq8"])
            extract_blockdiag(psq, asums, qav, "a")
            for k in range(NB):
                nc.vector.tensor_scalar_mul(wbq8[:, k, :], wbs[:, k, :],
                                            qav[:, k:k + 1])
            psw = fold16_dr(wbq8, w8sb["wkT8"])
            wsb16 = sp.tile([H, D], BF, tag="sb16", bufs=2)
            nc.vector.tensor_copy(wsb16[:], psw[:])
            transposes16(wsb16, wsbT8, 1.0)
            for t in range(8):
                e1_tile(t)

            # ============ Phase C: b-scores + b-pool + E1 tiles ============
            bpool_ps = ps_pool.tile([H, 2, SC], FP, tag="plps")
            hresc = {}
            for c in range(NSC):
                # prefetch D-phase weights (gpsimd is sacrificial: it may
                # block on the hres tile ring without stalling other work)
                nc.sync.dma_start(out=w8sb["wvT8"][:, c, :],
                                  in_=W8_d["wvT8"][:, c, :])
                w = sp.tile([P, D], BF, tag="hresc", bufs=4, name=f"hres{c}")
                nc.gpsimd.dma_start(out=w[:, 0:SC], in_=hres_d[:, c, 0:SC])
                nc.gpsimd.dma_start(out=w[:, SC:D], in_=hres_d[:, c, SC:D])
                hresc[c] = w
                score_chunk(c, wsbT8, SB * SW, bwT8, bsums)
                pool_chunk(c, bwT8, bpool_ps)
                e1_tile(8 + c)

            # ============ Phase C2: kav fold -> pav ============
            bwx16 = sp.tile([H, D], BF, tag="sb16", bufs=2)
            nc.vector.tensor_copy(bwx16[:], bpool_ps[:])
            transposes16(bwx16, bwxT8, 1.0 / SW)
            psk = fold16_dr(bwxT8, w8sb["wk8"])
            extract_blockdiag(psk, bsums, kv, "b")
            nc.vector.tensor_mul(pav[:], qav[:], kv[:])
            nc.vector.tensor_scalar_mul(pav65[:], pav[:], SM1 / SW)
            for t in range(16, NT):
                e1_tile(t)

        # ============ Phase D: Me8 = fp8(32*(Mtot + I) - up(M8h)) ========
        with tc.tile_pool(name="dtmp", bufs=1) as dtmp, \
             tc.tile_pool(name="ps_mm", bufs=2, space="PSUM") as ps_mm:
            m1T8 = dtmp.tile([P, NB, D], F8, tag="m1T8")
            for k in range(NB):
                nc.scalar.activation(m1T8[:, k, :], w8sb["wvT8"][:, k, :],
                                     AF.Copy, scale=pav65[:, k:k + 1])
            w18 = dtmp.tile([P, NB, D], F8, tag="w18")
            nc.sync.dma_start(out=w18[:, 0:4, :], in_=W8_d["w18"][:, 0:4, :])
            nc.scalar.dma_start(out=w18[:, 4:8, :], in_=W8_d["w18"][:, 4:8, :])
            psc = XN / (SM1 * SW)
            for m in range(NB):
                ps = ps_mm.tile([P, 2, SC], FP, tag="mm")
                for half in range(2):
                    for k2 in range(NB // 2):
                        nc.tensor.matmul(
                            ps[:, half, :],
                            m1T8[:, 2 * k2:2 * k2 + 2, m * P:(m + 1) * P],
                            w18[:, 2 * k2:2 * k2 + 2,
                                half * SC:(half + 1) * SC],
                            start=(k2 == 0), stop=(k2 == NB // 2 - 1),
                            perf_mode=DR)
                m32blk = sp.tile([P, D], BF, tag="m32blk", bufs=2)
                nc.vector.scalar_tensor_tensor(m32blk[:], ps[:], psc,
                                               hresc[m][:], op0=AL.mult,
                                               op1=AL.add)
                nc.scalar.activation(me8[:, m, :], m32blk[:], AF.Copy,
                                     scale=XE)

        # ====== Phase E2: 32*y = y12 + x8@Me8/32 ; LN epilogue ======
        with tc.tile_pool(name="e2sp", bufs=1) as e2sp, \
             tc.tile_pool(name="ps_e2", bufs=2, space="PSUM") as ps_e2:
            for t in range(NT):
                s0 = t * P
                if t >= NT - 8:
                    y12r = ykt[t]
                else:
                    y12r = e2sp.tile([P, D], BF, tag="y12r", bufs=6,
                                     name=f"y12r{t}")
                    nc.sync.dma_start(out=y12r[:, 0:SC],
                                      in_=y12_d.ap()[s0:s0 + P, 0:SC])
                    nc.scalar.dma_start(out=y12r[:, SC:D],
                                        in_=y12_d.ap()[s0:s0 + P, SC:D])
                pn3 = ps_e2.tile([P, 2, SC], FP, tag="e2")
                for half in range(2):
                    hsl = slice(half * SC, (half + 1) * SC)
                    for k2 in range(NB // 2):
                        nc.tensor.matmul(
                            pn3[:, half, :],
                            x8[:, 2 * k2:2 * k2 + 2, s0:s0 + P],
                            me8[:, 2 * k2:2 * k2 + 2, hsl],
                            start=(k2 == 0), stop=(k2 == NB // 2 - 1),
                            perf_mode=DR)
                ybf = sp.tile([P, D], BF, tag="ybf", bufs=3)
                sy = small.tile([P, 1], FP, tag="sy", bufs=4)
                nc.vector.scalar_tensor_tensor(ybf[:], pn3[:], 1.0 / XN,
                                               y12r[:], op0=AL.mult,
                                               op1=AL.add, accum_out=sy[:])
                sqs = sp.tile([P, D], BF, tag="sqs", bufs=2)
                s2 = small.tile([P, 1], FP, tag="s2", bufs=4)
                nc.scalar.activation(sqs[:], ybf[:], AF.Square,
                                     accum_out=s2[:])
                mu = small.tile([P, 1], FP, tag="mu", bufs=4)
                nc.vector.tensor_scalar_mul(mu[:], sy[:], 1.0 / D)
                mu2 = small.tile([P, 1], FP, tag="mu2", bufs=4)
                nc.vector.tensor_mul(mu2[:], mu[:], mu[:])
                var = small.tile([P, 1], FP, tag="var", bufs=4)
                nc.vector.scalar_tensor_tensor(var[:], s2[:], 1.0 / D,
                                               mu2[:], op0=AL.mult,
                                               op1=AL.subtract)
                sq = small.tile([P, 1], FP, tag="sq", bufs=4)
                nc.scalar.activation(sq[:], var[:], AF.Sqrt,
                                     bias=eps_t[:, :1], scale=1.0)
                rstd = small.tile([P, 1], FP, tag="rstd", bufs=4)
                nc.vector.reciprocal(rstd[:], sq[:])
                nmr = small.tile([P, 1], FP, tag="nmr", bufs=4)
                nc.vector.scalar_tensor_tensor(nmr[:], mu[:], -1.0,
                                               rstd[:], op0=AL.mult,
                                               op1=AL.mult)
                tb = sp.tile([P, D], BF, tag="tb", bufs=3)
                if t % 2 == 0:
                    nc.scalar.activation(tb[:], ybf[:], AF.Identity,
                                         bias=nmr[:, :1], scale=rstd[:, :1])
                else:
                    nc.gpsimd.tensor_scalar(tb[:], ybf[:], rstd[:, :1],
                                            nmr[:, :1], op0=AL.mult,
                                            op1=AL.add)
                oeng = (nc.sync, nc.scalar)[(t + 1) % 2]
                oeng.dma_start(out=out_d[s0:s0 + P, :], in_=tb[:])

    nc.compile()
    return nc


def _host_inputs_fast(x, Wq, Wk, Wv, Wa, Wb, Wu, Wo):
    import ml_dtypes
    BF = ml_dtypes.bfloat16
    F8 = ml_dtypes.float8_e4m3

    def lay(W):
        N = W.shape[1]
        return np.ascontiguousarray(
            W.reshape(NB, P, N).transpose(1, 0, 2)).astype(BF)

    def lay8(W, s):
        return lay(s * W).astype(F8)

    wqoI = lay(XN * (Wq @ Wo + np.eye(D, dtype=np.float32)))
    m8h = wqoI.astype(F8)
    hres16 = (wqoI.astype(np.float32) - m8h.astype(np.float32)).astype(BF)
    weights = {
        "wq8": lay8(Wq, SW),
        "wkT8": lay8(np.ascontiguousarray(Wk.T), SW),
        "wk8": lay8(Wk, SW),
        "wvT8": lay8(np.ascontiguousarray(Wv.T), SW),
        "w18": lay8(Wu @ Wo, SW),
        "m8h": m8h,
        "hres16": hres16,
        "waq8": lay8((Wq @ Wa), SCALE * SA),
        "wbs": lay(Wb * (SCALE * SB)),
    }

    def layx(a):
        return np.ascontiguousarray(a.reshape(S, NB, P).transpose(2, 1, 0))

    def per_core(xi):
        xb = xi.astype(BF)
        xbf = xb.astype(np.float32)
        x8n = xbf.astype(F8)
        # scale 1: subnormal-degraded residual, measured equivalent
        xe8n = (xbf - x8n.astype(np.float32)).astype(F8)
        m = {
            "xT8": layx(x8n),
            "xeT8": layx(xe8n),
            "xn8": (XN * xbf).astype(F8),
        }
        m.update(weights)
        return m

    return per_core


def _build_general():
    """Full-generality variant (arbitrary mask/bias/LN-affine values)."""
    import concourse.mybir as mybir
    import concourse.tile as tile
    from concourse import bacc
    from concourse.masks import make_identity
    from contextlib import ExitStack

    FP = mybir.dt.float32
    BF = mybir.dt.bfloat16
    F8 = mybir.dt.float8e4
    DR = mybir.MatmulPerfMode.DoubleRow
    AL = mybir.AluOpType
    AF = mybir.ActivationFunctionType

    SM1G = 65536.0
    SW1 = 64.0

    nc = bacc.Bacc("TRN2", target_bir_lowering=False, debug=False)

    xT8_d = nc.declare_dram_parameter("xT8", [P, NB, S], F8, isOutput=False)
    xeT8_d = nc.declare_dram_parameter("xeT8", [P, NB, S], F8, isOutput=False)
    xn8_d = nc.declare_dram_parameter("xn8", [S, D], F8, isOutput=False)
    mska_d = nc.declare_dram_parameter("mska", [1, S], BF, isOutput=False)
    mskb_d = nc.declare_dram_parameter("mskb", [1, S], BF, isOutput=False)
    W_d = {
        w: nc.declare_dram_parameter(w, [P, NB, D], BF, isOutput=False)
        for w in ("Wq", "WkT", "Wk", "WvT", "w164", "wqo32")
    }
    waq8_d = nc.declare_dram_parameter("waq8", [P, NB, H], F8, isOutput=False)
    wbs_d = nc.declare_dram_parameter("wbs", [P, NB, H], BF, isOutput=False)
    abias_d = nc.declare_dram_parameter("abias", [H, 1], FP, isOutput=False)
    bbs_d = nc.declare_dram_parameter("bbs", [H, 1], FP, isOutput=False)
    b_d = {
        b: nc.declare_dram_parameter(b, [P, NB], FP, isOutput=False)
        for b in ("bq", "bk", "bv")
    }
    hrow_d = nc.declare_dram_parameter("hrow", [1, D], BF, isOutput=False)
    lng_d = nc.declare_dram_parameter("lng16b", [P, D], BF, isOutput=False)
    lnb_d = nc.declare_dram_parameter("lnb16b", [P, D], BF, isOutput=False)
    out_d = nc.declare_dram_parameter("out", [S, D], BF, isOutput=True)

    xn8_r = xn8_d.ap().rearrange("(t p) f -> p t f", p=P)

    with tile.TileContext(nc) as tc, ExitStack() as ctx:
        consts = ctx.enter_context(tc.tile_pool(name="consts", bufs=1))
        wpers = ctx.enter_context(tc.tile_pool(name="wpers", bufs=1))
        sp = ctx.enter_context(tc.tile_pool(name="sp", bufs=2))
        small = ctx.enter_context(tc.tile_pool(name="small", bufs=2))

        id16 = consts.tile([H, H], BF, tag="id16")
        make_identity(nc, id16[:])
        id32 = consts.tile([P, P], BF, tag="id32")
        make_identity(nc, id32[:])
        nc.vector.tensor_scalar_mul(id32[:], id32[:], XN)
        ones16 = consts.tile([1, H], BF, tag="ones16")
        nc.vector.memset(ones16[:], 1.0)
        eps_t = consts.tile([P, 1], FP, tag="eps")
        nc.vector.memset(eps_t[:], EPS * XN * XN)

        waq8 = consts.tile([P, NB, H], F8, tag="waq8")
        nc.gpsimd.dma_start(out=waq8[:], in_=waq8_d[:])
        wbs = consts.tile([P, NB, H], BF, tag="wbs")
        nc.gpsimd.dma_start(out=wbs[:], in_=wbs_d[:])
        abias = consts.tile([H, 1], FP, tag="abias")
        nc.gpsimd.dma_start(out=abias[:], in_=abias_d[:])
        bbs = consts.tile([H, 1], FP, tag="bbs")
        nc.gpsimd.dma_start(out=bbs[:], in_=bbs_d[:])
        bias_t = {}
        for b in ("bq", "bk", "bv"):
            t = consts.tile([P, NB], FP, tag=f"b_{b}")
            nc.gpsimd.dma_start(out=t[:], in_=b_d[b][:])
            bias_t[b] = t
        bk16 = consts.tile([P, NB], BF, tag="bk16")
        nc.vector.tensor_copy(bk16[:], bias_t["bk"][:])
        lng_b = consts.tile([P, D], BF, tag="lng")
        nc.sync.dma_start(out=lng_b[:], in_=lng_d[:])
        lnb_b = consts.tile([P, D], BF, tag="lnb")
        nc.sync.dma_start(out=lnb_b[:], in_=lnb_d[:])
        hrow = consts.tile([1, D], BF, tag="hrow")
        nc.sync.dma_start(out=hrow[:], in_=hrow_d[:])

        x8 = wpers.tile([P, NB, S], F8, tag="x8")
        xe8 = wpers.tile([P, NB, S], F8, tag="xe8")
        xn8 = wpers.tile([P, NT, D], F8, tag="xn8")
        mtot8 = wpers.tile([P, NB, D], F8, tag="mtot8")
        me8 = wpers.tile([P, NB, D], F8, tag="me8", name="me8")
        awT8 = consts.tile([P, NT, H], F8, tag="awT8")
        bwT8 = consts.tile([P, NT, H], F8, tag="bwT8")
        asums = consts.tile([H, NSC], FP, tag="asums")
        bsums = consts.tile([H, NSC], FP, tag="bsums")
        qav = consts.tile([P, NB], FP, tag="qav")
        kv = consts.tile([P, NB], FP, tag="kv")
        pav = consts.tile([P, NB], FP, tag="pav")
        wbq = consts.tile([P, NB, H], BF, tag="wbq")
        awxT = consts.tile([P, NB, H], BF, tag="awxT")
        bwxT = consts.tile([P, NB, H], BF, tag="bwxT")
        wsbT8 = consts.tile([P, NB, H], F8, tag="wsbT8")
        biasb = consts.tile([H, 1], FP, tag="biasb")
        bvp16 = consts.tile([P, NB], BF, tag="bvp16")
        crow_b = consts.tile([P, D], BF, tag="crowb")

        with tc.tile_pool(name="ps_sc", bufs=2, space="PSUM") as ps_sc, \
             tc.tile_pool(name="ps_tp", bufs=2, space="PSUM") as ps_tp, \
             tc.tile_pool(name="ps_pool", bufs=1, space="PSUM") as ps_pool, \
             tc.tile_pool(name="ps_fold", bufs=1, space="PSUM") as ps_fold:

            def score_chunk(c, wstat8, msk_d, bias_s, sscale, wT_t, sums):
                lo = c * SC
                mc = sp.tile([1, SC], BF, tag="msk", bufs=2)
                nc.sync.dma_start(out=mc[:], in_=msk_d[:, lo:lo + SC])
                ps = ps_sc.tile([H, SC], FP, tag="sc")
                for k2 in range(NB // 2):
                    nc.tensor.matmul(ps[:], wstat8[:, 2 * k2:2 * k2 + 2, :],
                                     x8[:, 2 * k2:2 * k2 + 2, lo:lo + SC],
                                     start=(k2 == 0), stop=False,
                                     perf_mode=DR)
                nc.tensor.matmul(ps[:], ones16[:1, :], mc[:1, :],
                                 start=False, stop=True)
                awc = sp.tile([H, SC], BF, tag="wc", bufs=2)
                nc.scalar.activation(awc[:], ps[:], AF.Exp,
                                     bias=bias_s[:, :1], scale=1.0 / sscale,
                                     accum_out=sums[:, c:c + 1])
                for i in range(CPB):
                    tp = ps_tp.tile([P, H], BF, tag="tp")
                    nc.tensor.matmul(tp[:], awc[:, i * P:(i + 1) * P],
                                     id16[:, :], is_transpose=True)
                    nc.vector.tensor_copy(wT_t[:, c * CPB + i, :], tp[:])

            def pool_chunk(c, wT_t, pool_ps):
                for pr in (2 * c, 2 * c + 1):
                    for half in range(2):
                        nc.tensor.matmul(
                            pool_ps[:, half, :],
                            wT_t[:, 2 * pr:2 * pr + 2, :],
                            xn8[:, 2 * pr:2 * pr + 2,
                                half * SC:(half + 1) * SC],
                            start=(pr == 0), stop=(pr == NPAIR - 1),
                            perf_mode=DR, skip_group_check=True)

            def transpose16(src16, dst, j):
                tp = ps_tp.tile([P, H], BF, tag="tp")
                nc.tensor.matmul(tp[:], src16[:, j * P:(j + 1) * P],
                                 id16[:, :], is_transpose=True)
                nc.vector.tensor_copy(dst[:, j, :], tp[:])

            def fold16(srcT, wname, eng):
                psf = ps_fold.tile([H, 2, SC], FP, tag="fold")
                for k in range(NB):
                    wc = sp.tile([P, D], BF, tag="wfold", bufs=4)
                    eng.dma_start(out=wc[:], in_=W_d[wname][:, k, :])
                    for half in range(2):
                        nc.tensor.matmul(
                            psf[:, half, :], srcT[:, k, :],
                            wc[:, half * SC:(half + 1) * SC],
                            start=(k == 0), stop=(k == NB - 1),
                            skip_group_check=True)
                return psf

            def extract_blockdiag(pool_ps, sums, av_t, bias_av, wtag):
                tot = small.tile([H, 1], FP, tag=f"tot{wtag}")
                nc.vector.reduce_sum(tot[:], sums[:], axis=mybir.AxisListType.X)
                nc.vector.tensor_scalar_mul(tot[:], tot[:], XN)
                rinv = small.tile([H, 1], FP, tag=f"rinv{wtag}")
                nc.vector.reciprocal(rinv[:], tot[:])
                pool_sb = sp.tile([H, D], BF, tag="sb16", bufs=2)
                nc.vector.tensor_scalar_mul(pool_sb[:], pool_ps[:], rinv[:, :1])
                for j in range(NB):
                    tpp = ps_tp.tile([P, H], BF, tag="tp")
                    nc.tensor.matmul(tpp[:], pool_sb[:, j * P:(j + 1) * P],
                                     id16[:, :], is_transpose=True)
                    nc.vector.tensor_copy(
                        av_t[0:64, j:j + 1], tpp[0:64, 2 * j:2 * j + 1])
                    nc.vector.tensor_copy(
                        av_t[64:128, j:j + 1], tpp[64:128, 2 * j + 1:2 * j + 2])
                nc.vector.tensor_add(av_t[:], av_t[:], bias_av[:])

            apool_ps = ps_pool.tile([H, 2, SC], FP, tag="plps")
            for c in range(NSC):
                lo = c * SC
                nc.gpsimd.dma_start(out=x8[:, :, lo:lo + SC],
                                    in_=xT8_d[:, :, lo:lo + SC])
                nc.scalar.dma_start(out=xn8[:, 4 * c:4 * c + 4, :],
                                    in_=xn8_r[:, 4 * c:4 * c + 4, :])
                score_chunk(c, waq8, mska_d, abias, SA, awT8, asums)
                pool_chunk(c, awT8, apool_ps)
            for c in range(NSC):
                lo = c * SC
                eng = (nc.gpsimd, nc.scalar)[c % 2]
                eng.dma_start(out=xe8[:, :, lo:lo + SC],
                              in_=xeT8_d[:, :, lo:lo + SC])

            awx16 = sp.tile([H, D], BF, tag="sb16", bufs=2)
            nc.vector.tensor_copy(awx16[:], apool_ps[:])
            for j in range(NB):
                transpose16(awx16, awxT, j)
            psq = fold16(awxT, "Wq", nc.sync)
            extract_blockdiag(psq, asums, qav, bias_t["bq"], "a")
            for k in range(NB):
                nc.vector.tensor_scalar_mul(wbq[:, k, :], wbs[:, k, :],
                                            qav[:, k:k + 1])
            psw = fold16(wbq, "WkT", nc.sync)
            wsb16 = sp.tile([H, D], BF, tag="sb16", bufs=2)
            nc.vector.tensor_copy(wsb16[:], psw[:])
            for j in range(NB):
                transpose16(wsb16, wsbT8, j)
            psbb = ps_fold.tile([H, 1], FP, tag="fold", name="psbb")
            for k in range(NB):
                nc.tensor.matmul(psbb[:], wbq[:, k, :], bk16[:, k:k + 1],
                                 start=(k == 0), stop=(k == NB - 1))
            nc.vector.scalar_tensor_tensor(biasb[:], psbb[:], 1.0 / SB,
                                           bbs[:], op0=AL.mult, op1=AL.add)

            bpool_ps = ps_pool.tile([H, 2, SC], FP, tag="plps")
            for c in range(NSC):
                score_chunk(c, wsbT8, mskb_d, biasb, SB, bwT8, bsums)
                pool_chunk(c, bwT8, bpool_ps)

            bwx16 = sp.tile([H, D], BF, tag="sb16", bufs=2)
            nc.vector.tensor_copy(bwx16[:], bpool_ps[:])
            for j in range(NB):
                transpose16(bwx16, bwxT, j)
            psk = fold16(bwxT, "Wk", nc.sync)
            extract_blockdiag(psk, bsums, kv, bias_t["bk"], "b")
            nc.vector.tensor_mul(pav[:], qav[:], kv[:])
            bvp = small.tile([P, NB], FP, tag="bvp")
            nc.vector.tensor_mul(bvp[:], bias_t["bv"][:], pav[:])
            nc.vector.tensor_copy(bvp16[:], bvp[:])

        with tc.tile_pool(name="dtmp", bufs=1) as dtmp, \
             tc.tile_pool(name="ps_mm", bufs=2, space="PSUM") as ps_mm, \
             tc.tile_pool(name="ps_cr", bufs=1, space="PSUM") as ps_cr:
            pav65 = small.tile([P, NB], FP, tag="pav65")
            nc.vector.tensor_scalar_mul(pav65[:], pav[:], SM1G)
            m1T8 = dtmp.tile([P, NB, D], F8, tag="m1T8")
            for k in range(NB):
                wvc = sp.tile([P, D], BF, tag="wfold", bufs=4)
                nc.sync.dma_start(out=wvc[:], in_=W_d["WvT"][:, k, :])
                nc.scalar.activation(m1T8[:, k, :], wvc[:], AF.Copy,
                                     scale=pav65[:, k:k + 1])
            w18 = dtmp.tile([P, NB, D], F8, tag="w18")
            cr_ps = ps_cr.tile([H, 2, SC], FP, tag="cr")
            for k in range(NB):
                w1c = sp.tile([P, D], BF, tag="wfold", bufs=4)
                nc.sync.dma_start(out=w1c[:], in_=W_d["w164"][:, k, :])
                nc.scalar.activation(w18[:, k, :], w1c[:], AF.Copy)
                for half in range(2):
                    nc.tensor.matmul(
                        cr_ps[0:1, half, :], bvp16[:, k:k + 1],
                        w1c[:, half * SC:(half + 1) * SC],
                        start=(k == 0), stop=(k == NB - 1),
                        skip_group_check=True)
            crow16 = sp.tile([1, D], BF, tag="sb16", bufs=2)
            nc.vector.scalar_tensor_tensor(
                crow16[:1, :], cr_ps[0:1, :], XN / SW1, hrow[:1, :],
                op0=AL.mult, op1=AL.add)
            nc.gpsimd.partition_broadcast(crow_b[:], crow16[:1, :])

            psc = XN / (SM1G * SW1)
            for m in range(NB):
                ps = ps_mm.tile([P, 2, SC], FP, tag="mm")
                for half in range(2):
                    for k2 in range(NB // 2):
                        nc.tensor.matmul(
                            ps[:, half, :],
                            m1T8[:, 2 * k2:2 * k2 + 2, m * P:(m + 1) * P],
                            w18[:, 2 * k2:2 * k2 + 2,
                                half * SC:(half + 1) * SC],
                            start=(k2 == 0), stop=(k2 == NB // 2 - 1),
                            perf_mode=DR)
                wqoc = sp.tile([P, D], BF, tag="wqoc", bufs=2)
                nc.sync.dma_start(out=wqoc[:], in_=W_d["wqo32"][:, m, :])
                m32blk = sp.tile([P, D], BF, tag="m32blk", bufs=2)
                nc.vector.scalar_tensor_tensor(m32blk[:], ps[:], psc,
                                               wqoc[:], op0=AL.mult,
                                               op1=AL.add)
                nc.vector.tensor_add(m32blk[:, m * P:(m + 1) * P],
                                     m32blk[:, m * P:(m + 1) * P], id32[:])
                nc.scalar.activation(mtot8[:, m, :], m32blk[:], AF.Copy)
                up16 = sp.tile([P, D], BF, tag="up16", bufs=2)
                nc.vector.tensor_copy(up16[:], mtot8[:, m, :])
                me32 = sp.tile([P, D], BF, tag="me32", bufs=2)
                nc.vector.tensor_sub(me32[:], m32blk[:], up16[:])
                nc.scalar.activation(me8[:, m, :], me32[:], AF.Copy,
                                     scale=XE)

        with tc.tile_pool(name="ps_e1", bufs=2, space="PSUM") as ps_e1, \
             tc.tile_pool(name="ps_e2", bufs=2, space="PSUM") as ps_e2:
            for t in range(NT):
                s0 = t * P
                pn1 = ps_e1.tile([P, 2, SC], FP, tag="e1")
                pn2 = ps_e2.tile([P, 2, SC], FP, tag="e2")
                for half in range(2):
                    hsl = slice(half * SC, (half + 1) * SC)
                    for k2 in range(NB // 2):
                        nc.tensor.matmul(
                            pn1[:, half, :],
                            x8[:, 2 * k2:2 * k2 + 2, s0:s0 + P],
                            mtot8[:, 2 * k2:2 * k2 + 2, hsl],
                            start=(k2 == 0), stop=(k2 == NB // 2 - 1),
                            perf_mode=DR)
                    for k2 in range(NB // 2):
                        nc.tensor.matmul(
                            pn2[:, half, :],
                            xe8[:, 2 * k2:2 * k2 + 2, s0:s0 + P],
                            mtot8[:, 2 * k2:2 * k2 + 2, hsl],
                            start=(k2 == 0), stop=False, perf_mode=DR)
                    for k2 in range(NB // 2):
                        nc.tensor.matmul(
                            pn2[:, half, :],
                            x8[:, 2 * k2:2 * k2 + 2, s0:s0 + P],
                            me8[:, 2 * k2:2 * k2 + 2, hsl],
                            start=False, stop=(k2 == NB // 2 - 1),
                            perf_mode=DR)
                t28 = sp.tile([P, D], BF, tag="t28", bufs=2)
                nc.scalar.activation(t28[:], pn2[:], AF.Copy,
                                     scale=1.0 / XN)
                nc.vector.tensor_add(t28[:], t28[:], crow_b[:])
                ybf = sp.tile([P, D], BF, tag="ybf", bufs=2)
                nc.vector.scalar_tensor_tensor(ybf[:], pn1[:], 1.0,
                                               t28[:], op0=AL.mult,
                                               op1=AL.add)
                stats = small.tile([P, 2, 6], FP, tag="stats")
                nc.vector.bn_stats(stats[:, 0, :], ybf[:, 0:SC])
                nc.vector.bn_stats(stats[:, 1, :], ybf[:, SC:D])
                mv = small.tile([P, 2], FP, tag="mv")
                nc.vector.bn_aggr(mv[:], stats[:])
                sq = small.tile([P, 1], FP, tag="sq")
                nc.scalar.activation(sq[:], mv[:, 1:2], AF.Sqrt,
                                     bias=eps_t[:, :1], scale=1.0)
                rstd = small.tile([P, 1], FP, tag="rstd")
                nc.vector.reciprocal(rstd[:], sq[:])
                nmr = small.tile([P, 1], FP, tag="nmr")
                nc.vector.scalar_tensor_tensor(nmr[:], mv[:, 0:1], -1.0,
                                               rstd[:], op0=AL.mult,
                                               op1=AL.mult)
                tb = sp.tile([P, D], BF, tag="tb", bufs=3)
                nc.scalar.activation(tb[:], ybf[:], AF.Identity,
                                     bias=nmr[:, :1], scale=rstd[:, :1])
                nc.vector.tensor_mul(tb[:], tb[:], lng_b[:])
                nc.gpsimd.tensor_add(tb[:], tb[:], lnb_b[:])
                oeng = (nc.sync, nc.scalar)[(t + 1) % 2]
                oeng.dma_start(out=out_d[s0:s0 + P, :], in_=tb[:])

    nc.compile()
    return nc


def _install_ntff_hook_shim():
    """The agent image's antenv lacks axon_hooks, so trace=True degrades.
    Recreate the hook from the boot helper so neuron-profile works."""
    import types
    try:
        import antenv.axon_hooks  # noqa: F401
        return
    except ImportError:
        pass
    try:
        import antenv
        from trn_agent_boot.trn_boot import _ntff_profile_via_ctypes
        hook = _ntff_profile_via_ctypes("/opt/axon/libaxon_pjrt.so")
        mod = types.ModuleType("antenv.axon_hooks")
        mod._hook = hook
        mod.get_axon_ntff_profile_hook = lambda: mod._hook
        mod.set_axon_ntff_profile_hook = lambda h: setattr(mod, "_hook", h)
        sys.modules["antenv.axon_hooks"] = mod
        antenv.axon_hooks = mod
    except Exception as e:  # tracing is best-effort
        print(f"ntff hook shim failed: {e}", file=sys.stderr)


def kernel(x, mask, Wq, bq, Wk, bk, Wv, bv, Wa, ba, Wb, bb, Wu, bu, Wo, bo,
           ln_g, ln_b):
    global LAST_EXEC_TIME_NS
    import ml_dtypes
    from concourse.bass_utils import run_bass_kernel_spmd

    BF = ml_dtypes.bfloat16
    F8 = ml_dtypes.float8_e4m3
    f32 = lambda a: np.ascontiguousarray(np.asarray(a, dtype=np.float32))

    x = f32(x)
    B = x.shape[0]
    assert B == NCORES and x.shape == (B, S, D)
    mask = f32(mask).reshape(B, S)
    Wq, Wk, Wv, Wu, Wo = f32(Wq), f32(Wk), f32(Wv), f32(Wu), f32(Wo)
    Wa, Wb = f32(Wa), f32(Wb)
    bq, bk, bv, ba, bb, bu, bo = map(f32, (bq, bk, bv, ba, bb, bu, bo))
    ln_g, ln_b = f32(ln_g), f32(ln_b)

    plain = (not mask.any()) and all(
        not a.any() for a in (bq, bk, bv, ba, bb, bu, bo, ln_b)) and bool(
        np.all(ln_g == 1.0))

    trace = bool(int(os.environ.get("KERNEL_TRACE", "0")))
    if trace:
        _install_ntff_hook_shim()

    if plain:
        if "fast" not in _COMPILED:
            _COMPILED["fast"] = _build_fast()
        nc = _COMPILED["fast"]
        per_core = _host_inputs_fast(x, Wq, Wk, Wv, Wa, Wb, Wu, Wo)
        in_maps = [per_core(x[i]) for i in range(B)]
    else:
        if "gen" not in _COMPILED:
            _COMPILED["gen"] = _build_general()
        nc = _COMPILED["gen"]

        def lay(W):
            N = W.shape[1]
            return np.ascontiguousarray(
                W.reshape(NB, P, N).transpose(1, 0, 2)).astype(BF)

        W1f = Wu @ Wo
        weights = {
            "Wq": lay(Wq), "WkT": lay(np.ascontiguousarray(Wk.T)),
            "Wk": lay(Wk), "WvT": lay(np.ascontiguousarray(Wv.T)),
            "w164": lay(64.0 * W1f), "wqo32": lay(XN * (Wq @ Wo)),
            "waq8": lay((Wq @ Wa) * (SCALE * SA)).astype(F8),
            "wbs": lay(Wb * (SCALE * SB)),
        }
        smalls = {
            "abias": (((bq @ Wa) + ba) * SCALE).reshape(H, 1),
            "bbs": (bb * SCALE).reshape(H, 1),
            "bq": np.ascontiguousarray(bq.reshape(NB, P).T),
            "bk": np.ascontiguousarray(bk.reshape(NB, P).T),
            "bv": np.ascontiguousarray(bv.reshape(NB, P).T),
            "hrow": (XN * ((bq + bu) @ Wo + bo)).reshape(1, D).astype(BF),
            "lng16b": np.ascontiguousarray(
                np.broadcast_to(ln_g.reshape(1, D), (P, D))).astype(BF),
            "lnb16b": np.ascontiguousarray(
                np.broadcast_to(ln_b.reshape(1, D), (P, D))).astype(BF),
        }

        def layx(a):
            return np.ascontiguousarray(
                a.reshape(S, NB, P).transpose(2, 1, 0))

        in_maps = []
        for i in range(B):
            xb = x[i].astype(BF)
            xbf = xb.astype(np.float32)
            x8n = xbf.astype(F8)
            xe8n = ((xbf - x8n.astype(np.float32)) * XE).astype(F8)
            m = {
                "xT8": layx(x8n),
                "xeT8": layx(xe8n),
                "xn8": (XN * xbf).astype(F8),
                "mska": (SA * mask[i:i + 1]).astype(BF),
                "mskb": (SB * mask[i:i + 1]).astype(BF),
            }
            m.update(weights)
            m.update(smalls)
            in_maps.append(m)

    res = run_bass_kernel_spmd(nc, in_maps, core_ids=list(range(NCORES)),
                               trace=trace)
    LAST_EXEC_TIME_NS = res.exec_time_ns
    out = np.stack([np.asarray(res.results[i]["out"]).astype(np.float32)
                    for i in range(B)], axis=0)
    return out


if __name__ == "__main__":
    np.random.seed(0)
    ins = {
        "x": np.random.randn(NCORES, S, D).astype(np.float32),
        "mask": np.zeros((NCORES, 1, S), np.float32),
    }
    std = 0.02
    for n, shp in (("Wq", (D, D)), ("Wk", (D, D)), ("Wv", (D, D)),
                   ("Wa", (D, H)), ("Wb", (D, H)), ("Wu", (D, D)),
                   ("Wo", (D, D))):
        ins[n] = (std * np.random.randn(*shp)).astype(np.float32)
    for n, shp in (("bq", (D,)), ("bk", (D,)), ("bv", (D,)), ("ba", (H,)),
                   ("bb", (H,)), ("bu", (D,)), ("bo", (D,)), ("ln_b", (D,))):
        ins[n] = np.zeros(shp, np.float32)
    ins["ln_g"] = np.ones((D,), np.float32)
    out = kernel(**ins)
    print("out", out.shape, out.dtype, float(np.abs(out).mean()))
